# revision 1
# baseline (speedup 1.0000x reference)
"""Trainium2 Bass kernel for nn_AxispoolingMamba.

Sharding: 8 cores = (batch b in 0..3) x (h-half in 0..1).
Each core gets x0[b, :, half*128:(half+1)*128, :]  ([256c, 128h, 256w]).
  Stage A: partial mean over w  -> pair AllGather -> full x_h[b]
  model1_h (replicated within pair, b-sharded across pairs)
  Stage C: gate by xm_h (own h rows) + partial sum over h -> pair AllReduce
  model1_w
  Stage D: out = xm_w * x0  (own h rows) -> per-core output shard.

Layout convention on chip: channel dim on partitions (tiles of 128),
sequence dim l on the free axis.  Selective scan uses the DVE
tensor_tensor_scan instruction: state = aexp[t]*state + dBu[t].
"""

import sys

sys.path.insert(0, "/opt/trn_rl_repo")

from contextlib import ExitStack  # noqa: E402

import numpy as np  # noqa: E402

import concourse.bass as bass  # noqa: E402
import concourse.bacc as bacc  # noqa: E402
import concourse.mybir as mybir  # noqa: E402
import concourse.tile as tile  # noqa: E402

F32 = mybir.dt.float32
AF = mybir.ActivationFunctionType
OP = mybir.AluOpType

D_MODEL = 256
D_INNER = 512
D_STATE = 16
DT_RANK = 16
D_CONV = 4
DEPTH = 2
L = 256          # sequence length for both mamba passes (h or w)
HLOC = 128       # h rows owned by one core
NMT_IN = 2 * D_INNER // 128   # 8
NDT = D_INNER // 128          # 4
NCT = D_MODEL // 128          # 2


def _block(nc, tc, ctx, P, i, x):
    """One mamba block. x: sbuf tile [128, NCT, L] (c-major). Returns same shape."""
    ap = P["act"]
    sp = P["scan"]
    pp = P["psum"]

    W_in, W_xp, W_dt, W_out = P["W_in"][i], P["W_xp"][i], P["W_dt"][i], P["W_out"][i]
    cw, cb, dtb, nA, Dpar = P["cw"][i], P["cb"][i], P["dtb"][i], P["nA"][i], P["Dp"][i]
    ones1 = P["ones1"]

    # ---- in_proj: xr[1024, L] = in_w @ x ----
    xx = ap.tile([128, NDT, L + D_CONV - 1], F32, tag="xx")   # left-pad 3 for conv
    res = ap.tile([128, NDT, L], F32, tag="res")
    nc.vector.memset(xx[:, :, 0:D_CONV - 1], 0.0)
    for mt in range(NMT_IN):
        ps = pp.tile([128, L], F32, tag="ps")
        for ct in range(NCT):
            nc.tensor.matmul(ps[:], W_in[:, ct, mt * 128:(mt + 1) * 128],
                             x[:, ct, :], start=(ct == 0), stop=(ct == NCT - 1))
        if mt < NDT:
            nc.scalar.activation(xx[:, mt, D_CONV - 1:], ps[:], AF.Copy)
        else:
            nc.scalar.activation(res[:, mt - NDT, :], ps[:], AF.Copy)

    # ---- causal depthwise conv + bias + silu ----
    u = ap.tile([128, NDT, L], F32, tag="u")
    cacc = ap.tile([128, NDT, L], F32, tag="cacc")
    for dt in range(NDT):
        nc.vector.tensor_scalar_mul(cacc[:, dt, :], xx[:, dt, 0:L], cw[:, dt, 0:1])
        for j in range(1, D_CONV):
            nc.vector.scalar_tensor_tensor(cacc[:, dt, :], xx[:, dt, j:j + L],
                                           cw[:, dt, j:j + 1], cacc[:, dt, :],
                                           OP.mult, OP.add)
        nc.scalar.activation(u[:, dt, :], cacc[:, dt, :], AF.Silu,
                             bias=cb[:, dt, :], scale=1.0)

    # ---- x_dbl = xproj @ u : [48, L] ----
    ps2 = pp.tile([48, L], F32, tag="ps48")
    for dt in range(NDT):
        nc.tensor.matmul(ps2[:], W_xp[:, dt, :], u[:, dt, :],
                         start=(dt == 0), stop=(dt == NDT - 1))
    xdbl = ap.tile([48, L], F32, tag="xdbl")
    nc.vector.tensor_copy(xdbl[:], ps2[:])

    # ---- delta = softplus(dt_w @ delta_r + dt_b) : [512, L] ----
    delta = ap.tile([128, NDT, L], F32, tag="cacc")  # reuse cacc slot
    for dt in range(NDT):
        ps3 = pp.tile([128, L], F32, tag="ps")
        nc.tensor.matmul(ps3[:], W_dt[:, dt * 128:(dt + 1) * 128],
                         xdbl[0:DT_RANK, :], start=True, stop=True)
        # softplus(v) = ln(1 + exp(v)), v = raw + dt_b
        nc.scalar.activation(delta[:, dt, :], ps3[:], AF.Exp,
                             bias=dtb[:, dt, :], scale=1.0)
        nc.vector.tensor_scalar_add(delta[:, dt, :], delta[:, dt, :], 1.0)
        nc.scalar.activation(delta[:, dt, :], delta[:, dt, :], AF.Ln)

    # ---- broadcast B, C across partitions: [128, 16, L] ----
    # flatten [16, L] -> [1, 16*L] via DMA, then ones[1,128].T @ flat chunks
    Bc = ap.tile([128, D_STATE, L], F32, tag="Bc")
    Cc = ap.tile([128, D_STATE, L], F32, tag="Cc")
    for t, base in ((Bc, DT_RANK), (Cc, DT_RANK + D_STATE)):
        bc_flat = ap.tile([1, D_STATE * L], F32, tag="bcflat")
        nc.sync.dma_start(bc_flat[:], xdbl[base:base + D_STATE, :])
        for ch in range(D_STATE * L // 512):
            ps4 = pp.tile([128, 512], F32, tag="ps512")
            nc.tensor.matmul(ps4[:], ones1[:], bc_flat[0:1, ch * 512:(ch + 1) * 512],
                             start=True, stop=True)
            nc.scalar.activation(
                t[:, 2 * ch:2 * ch + 2, :].rearrange("p n l -> p (n l)"), ps4[:],
                AF.Copy)

    # ---- du = delta * u ----
    du = ap.tile([128, NDT, L], F32, tag="du")
    nc.vector.tensor_mul(du[:], delta[:], u[:])

    # ---- selective scan per d-tile ----
    y = ap.tile([128, NDT, L], F32, tag="y")
    for dt in range(NDT):
        aexp = sp.tile([128, D_STATE, L], F32, tag="aexp")
        dbu = sp.tile([128, D_STATE, L], F32, tag="dbu")
        hh = sp.tile([128, D_STATE, L], F32, tag="hh")
        for n in range(D_STATE):
            nc.scalar.activation(aexp[:, n, :], delta[:, dt, :], AF.Exp,
                                 scale=nA[:, dt, n:n + 1])
        nc.vector.tensor_mul(dbu[:], du[:, dt:dt + 1, :].broadcast_to([128, D_STATE, L]),
                             Bc[:])
        for n in range(D_STATE):
            nc.vector.tensor_tensor_scan(hh[:, n, :], aexp[:, n, :], dbu[:, n, :],
                                         0.0, OP.mult, OP.add)
        hc = aexp  # reuse buffer
        nc.vector.tensor_mul(hc[:], hh[:], Cc[:])
        nc.vector.tensor_reduce(y[:, dt, :], hc[:].rearrange("p n l -> p l n"),
                                axis=mybir.AxisListType.X, op=OP.add)

    # ---- y = (y + u*D) * silu(res); out_proj ----
    for dt in range(NDT):
        nc.vector.scalar_tensor_tensor(y[:, dt, :], u[:, dt, :], Dpar[:, dt, :],
                                       y[:, dt, :], OP.mult, OP.add)
    nc.scalar.activation(res[:], res[:], AF.Silu)
    nc.vector.tensor_mul(y[:], y[:], res[:])

    xo = ap.tile([128, NCT, L], F32, tag="xo")
    for mt in range(NCT):
        ps5 = pp.tile([128, L], F32, tag="ps")
        for dt in range(NDT):
            nc.tensor.matmul(ps5[:], W_out[:, dt, mt * 128:(mt + 1) * 128],
                             y[:, dt, :], start=(dt == 0), stop=(dt == NDT - 1))
        nc.vector.tensor_copy(xo[:, mt, :], ps5[:])
    return xo


def _model1(nc, tc, ctx, P, x):
    for i in range(DEPTH):
        x = _block(nc, tc, ctx, P, i, x)
    return x


HCH = 8           # h rows per streaming chunk
NHC = HLOC // HCH  # 16 chunks


def build(n_cores=8, fake_pair=False):
    nc = bacc.Bacc(None, target_bir_lowering=False)
    nc.num_devices = n_cores

    x0s = nc.dram_tensor("x0s", [D_MODEL, HLOC, 256], F32, kind="ExternalInput")
    w_in = nc.dram_tensor("w_in_t", [DEPTH, D_MODEL, 2 * D_INNER], F32, kind="ExternalInput")
    w_xp = nc.dram_tensor("w_xp_t", [DEPTH, D_INNER, 48], F32, kind="ExternalInput")
    w_dt = nc.dram_tensor("w_dt_t", [DEPTH, DT_RANK, D_INNER], F32, kind="ExternalInput")
    w_out = nc.dram_tensor("w_out_t", [DEPTH, D_INNER, D_MODEL], F32, kind="ExternalInput")
    cw_d = nc.dram_tensor("conv_w_r", [DEPTH, D_INNER, D_CONV], F32, kind="ExternalInput")
    cb_d = nc.dram_tensor("conv_b", [DEPTH, D_INNER], F32, kind="ExternalInput")
    dtb_d = nc.dram_tensor("dt_b", [DEPTH, D_INNER], F32, kind="ExternalInput")
    nA_d = nc.dram_tensor("neg_a", [DEPTH, D_INNER, D_STATE], F32, kind="ExternalInput")
    Dp_d = nc.dram_tensor("d_par", [DEPTH, D_INNER], F32, kind="ExternalInput")
    hsel_d = nc.dram_tensor("hsel", [128, 2], F32, kind="ExternalInput")
    out_d = nc.dram_tensor("out", [D_MODEL, HLOC, 256], F32, kind="ExternalOutput")

    with tile.TileContext(nc) as tc, ExitStack() as ctx:
        wp = ctx.enter_context(tc.tile_pool(name="weights", bufs=1))
        ap = ctx.enter_context(tc.tile_pool(name="act", bufs=1))
        sp = ctx.enter_context(tc.tile_pool(name="scan", bufs=1))
        stp = ctx.enter_context(tc.tile_pool(name="stage", bufs=3))
        stpo = ctx.enter_context(tc.tile_pool(name="stageout", bufs=2))
        pp = ctx.enter_context(tc.tile_pool(name="psum", bufs=2, space="PSUM"))
        dp = ctx.enter_context(tc.tile_pool(name="dram", bufs=1, space="DRAM"))

        P = {"act": ap, "scan": sp, "psum": pp,
             "W_in": [], "W_xp": [], "W_dt": [], "W_out": [],
             "cw": [], "cb": [], "dtb": [], "nA": [], "Dp": []}
        for i in range(DEPTH):
            wi = wp.tile([128, NCT, 2 * D_INNER], F32, tag=f"win{i}")
            for ct in range(NCT):
                nc.sync.dma_start(wi[:, ct, :], w_in[i, ct * 128:(ct + 1) * 128, :])
            P["W_in"].append(wi)
            wx = wp.tile([128, NDT, 48], F32, tag=f"wxp{i}")
            wo = wp.tile([128, NDT, D_MODEL], F32, tag=f"wout{i}")
            cwt = wp.tile([128, NDT, D_CONV], F32, tag=f"cw{i}")
            cbt = wp.tile([128, NDT, 1], F32, tag=f"cb{i}")
            dtbt = wp.tile([128, NDT, 1], F32, tag=f"dtb{i}")
            nAt = wp.tile([128, NDT, D_STATE], F32, tag=f"na{i}")
            dpt = wp.tile([128, NDT, 1], F32, tag=f"dp{i}")
            for dt in range(NDT):
                sl = slice(dt * 128, (dt + 1) * 128)
                nc.sync.dma_start(wx[:, dt, :], w_xp[i, sl, :])
                nc.sync.dma_start(wo[:, dt, :], w_out[i, sl, :])
                nc.sync.dma_start(cwt[:, dt, :], cw_d[i, sl, :])
                nc.sync.dma_start(cbt[:, dt, :], cb_d[i, sl][:, None])
                nc.sync.dma_start(dtbt[:, dt, :], dtb_d[i, sl][:, None])
                nc.sync.dma_start(nAt[:, dt, :], nA_d[i, sl, :])
                nc.sync.dma_start(dpt[:, dt, :], Dp_d[i, sl][:, None])
            wd = wp.tile([DT_RANK, D_INNER], F32, tag=f"wdt{i}")
            nc.sync.dma_start(wd[:], w_dt[i])
            P["W_xp"].append(wx); P["W_out"].append(wo); P["W_dt"].append(wd)
            P["cw"].append(cwt); P["cb"].append(cbt); P["dtb"].append(dtbt)
            P["nA"].append(nAt); P["Dp"].append(dpt)
        ones1 = wp.tile([1, 128], F32, tag="ones1")
        nc.vector.memset(ones1[:], 1.0)
        P["ones1"] = ones1
        hsel = wp.tile([128, 2], F32, tag="hsel")
        nc.sync.dma_start(hsel[:], hsel_d[:])

        # ================= Stage A: partial sum over w =================
        xh_part = ap.tile([128, NCT, HLOC], F32, tag="xh_part")
        for ct in range(NCT):
            for hc in range(NHC):
                t = stp.tile([128, HCH, 256], F32, tag="x0chunk")
                nc.sync.dma_start(t[:], x0s[ct * 128:(ct + 1) * 128,
                                             hc * HCH:(hc + 1) * HCH, :])
                nc.vector.tensor_reduce(xh_part[:, ct, hc * HCH:(hc + 1) * HCH],
                                        t[:], axis=mybir.AxisListType.X, op=OP.add)

        # ================= Exchange 1: pair AllGather =================
        xh_full = ap.tile([128, NCT, L], F32, tag="xh_full")
        gin = dp.tile([128, NCT, HLOC], F32)
        gout = dp.tile([2, 128, NCT, HLOC], F32)
        nc.sync.dma_start(gin[:], xh_part[:])
        if fake_pair:
            nc.sync.dma_start(gout[0], gin[:])
            nc.sync.dma_start(gout[1], gin[:])
        else:
            groups = [[2 * b, 2 * b + 1] for b in range(n_cores // 2)]
            nc.gpsimd.collective_compute(
                "AllGather", OP.bypass, replica_groups=groups,
                ins=[gin.opt()], outs=[gout.opt()])
        for ct in range(NCT):
            for half in range(2):
                nc.sync.dma_start(xh_full[:, ct, half * HLOC:(half + 1) * HLOC],
                                  gout[half, :, ct, :])

        # ================= model1 over h =================
        xmh = _model1(nc, tc, ctx, P, xh_full)

        # gate rows for my h-half: gate[c, hloc] (select half via hsel one-hot)
        gate = ap.tile([128, NCT, HLOC], F32, tag="gate")
        for ct in range(NCT):
            nc.vector.tensor_scalar_mul(gate[:, ct, :], xmh[:, ct, 0:HLOC],
                                        hsel[:, 0:1])
            nc.vector.scalar_tensor_tensor(gate[:, ct, :], xmh[:, ct, HLOC:],
                                           hsel[:, 1:2], gate[:, ct, :],
                                           OP.mult, OP.add)

        # ================= Stage C: gated partial sum over h =================
        xw_part = ap.tile([128, NCT, 256], F32, tag="xw_part")
        for ct in range(NCT):
            for hc in range(NHC):
                t = stp.tile([128, HCH, 256], F32, tag="x0chunk")
                nc.sync.dma_start(t[:], x0s[ct * 128:(ct + 1) * 128,
                                             hc * HCH:(hc + 1) * HCH, :])
                for hi in range(HCH):
                    h = hc * HCH + hi
                    if h == 0:
                        nc.vector.tensor_scalar_mul(xw_part[:, ct, :], t[:, hi, :],
                                                    gate[:, ct, h:h + 1])
                    else:
                        nc.vector.scalar_tensor_tensor(xw_part[:, ct, :], t[:, hi, :],
                                                       gate[:, ct, h:h + 1],
                                                       xw_part[:, ct, :],
                                                       OP.mult, OP.add)

        # ================= Exchange 2: pair AllReduce =================
        xw = ap.tile([128, NCT, 256], F32, tag="xw")
        rin = dp.tile([128, NCT, 256], F32)
        rout = dp.tile([128, NCT, 256], F32)
        nc.sync.dma_start(rin[:], xw_part[:])
        if fake_pair:
            nc.sync.dma_start(rout[:], rin[:])
        else:
            groups = [[2 * b, 2 * b + 1] for b in range(n_cores // 2)]
            nc.gpsimd.collective_compute(
                "AllReduce", OP.add, replica_groups=groups,
                ins=[rin.opt()], outs=[rout.opt()])
        nc.sync.dma_start(xw[:], rout[:])

        # ================= model1 over w =================
        xmw = _model1(nc, tc, ctx, P, xw)

        # ================= Stage D: out = xmw (bcast over h) * x0 =================
        for ct in range(NCT):
            for hc in range(NHC):
                t = stp.tile([128, HCH, 256], F32, tag="x0chunk")
                o = stpo.tile([128, HCH, 256], F32, tag="ochunk")
                nc.sync.dma_start(t[:], x0s[ct * 128:(ct + 1) * 128,
                                             hc * HCH:(hc + 1) * HCH, :])
                nc.vector.tensor_mul(
                    o[:], t[:],
                    xmw[:, ct:ct + 1, :].broadcast_to([128, HCH, 256]))
                nc.sync.dma_start(out_d[ct * 128:(ct + 1) * 128,
                                        hc * HCH:(hc + 1) * HCH, :], o[:])

    nc.compile()
    return nc


def _prep_host(inputs):
    x0 = np.ascontiguousarray(inputs["x0"], dtype=np.float32)
    in_w = np.asarray(inputs["in_w"], np.float32)
    conv_w = np.asarray(inputs["conv_w"], np.float32)
    conv_b = np.asarray(inputs["conv_b"], np.float32)
    xproj_w = np.asarray(inputs["xproj_w"], np.float32)
    dt_w = np.asarray(inputs["dt_w"], np.float32)
    dt_b = np.asarray(inputs["dt_b"], np.float32)
    A_log = np.asarray(inputs["A_log"], np.float32)
    Dp = np.asarray(inputs["Dp"], np.float32)
    out_w = np.asarray(inputs["out_w"], np.float32)

    w = {}
    # fold the 1/256 pooling mean (exact power of two) into depth-0 in_proj
    w_in_t = np.ascontiguousarray(in_w.transpose(0, 2, 1))
    w_in_t[0] = w_in_t[0] * np.float32(2.0 ** -8)
    w["w_in_t"] = w_in_t
    w["w_xp_t"] = np.ascontiguousarray(xproj_w.transpose(0, 2, 1))
    w["w_dt_t"] = np.ascontiguousarray(dt_w.transpose(0, 2, 1))
    w["w_out_t"] = np.ascontiguousarray(out_w.transpose(0, 2, 1))
    w["conv_w_r"] = np.ascontiguousarray(conv_w[:, :, 0, :])
    w["conv_b"] = conv_b
    w["dt_b"] = dt_b
    w["neg_a"] = -np.exp(A_log)
    w["d_par"] = Dp
    return x0, w


def kernel(**inputs):
    from concourse.bass_utils import run_bass_kernel_spmd

    x0, w = _prep_host(inputs)
    nc = build(n_cores=8)

    in_maps = []
    for k in range(8):
        b, half = k // 2, k % 2
        m = dict(w)
        m["x0s"] = np.ascontiguousarray(x0[b, :, half * 128:(half + 1) * 128, :])
        hs = np.zeros((128, 2), np.float32)
        hs[:, half] = 1.0
        m["hsel"] = hs
        in_maps.append(m)

    res = run_bass_kernel_spmd(nc, in_maps, core_ids=list(range(8)))
    out = np.empty((4, 256, 256, 256), np.float32)
    for k in range(8):
        b, half = k // 2, k % 2
        out[b, :, half * 128:(half + 1) * 128, :] = res.results[k]["out"]
    return out



# revision 2
# speedup vs baseline: 1.4713x; 1.4713x over previous
"""Trainium2 Bass kernel for nn_AxispoolingMamba — optimized v2.

Sharding: 8 cores = (batch b in 0..3) x (h-half in 0..1).
Each core owns x0[b, :, half*128:(half+1)*128, :]  ([256c, 128h, 256w]).

Key optimizations over the v1 baseline:
  - x0 shard is converted to bf16 on first read and cached in SBUF
    (128 KB/partition), eliminating the HBM re-reads in stages C and D.
  - bf16 throughout (PE 1 cyc/row; DVE 2x for packed bf16 tensor_tensor,
    4x for tensor_scalar).
  - Selective scan via tensor_tensor_scan over flattened (n l); per-state
    restart forced by dA[:, :, l=0] = -200 (exp -> 0 carry-in).
  - Only Exp/Ln/Copy activations (single table set, no reload thrash);
    softplus = Ln(exp(x)+1), silu = x/(1+exp(-x)) with a DVE divide.
  - dA products on the Pool engine; stage-A/D work split DVE/ACT/Pool.
  - SBUF pool overlays: the 32 KB streaming buffers (stages A/D) share
    address space with the ~45 KB of model-internal tiles via nested
    tile pools, which pays for 16-row DMA chunks (transfer-bound DMA).
  - Collectives in bf16; exchange 2 is AllGather + local add.

Numerics: true-recurrence scan (like the v1 baseline); at the staged
input scale the whole model underflows fp32 to exactly zero, matching
the reference bit-for-bit; at 100x input scale both formulations agree
to ~1.5e-3 relative.
"""

import sys

sys.path.insert(0, "/opt/trn_rl_repo")

from contextlib import ExitStack  # noqa: E402

import numpy as np  # noqa: E402

import concourse.bass as bass  # noqa: E402
import concourse.bacc as bacc  # noqa: E402
import concourse.mybir as mybir  # noqa: E402
import concourse.tile as tile  # noqa: E402

F32 = mybir.dt.float32
BF16 = mybir.dt.bfloat16
AF = mybir.ActivationFunctionType
OP = mybir.AluOpType

D_MODEL = 256
D_INNER = 512
D_STATE = 16
DT_RANK = 16
D_CONV = 4
DEPTH = 2
L = 256          # sequence length for both mamba passes (h or w)
HLOC = 128       # h rows owned by one core
NDT = D_INNER // 128          # 4 d-tiles
NCT = D_MODEL // 128          # 2 c-tiles
ACH = 16                      # h rows per streaming chunk (stages A and D)
NAC = HLOC // ACH             # 8 chunks per ct


def _block(nc, P, i, x):
    """One mamba block. x: sbuf bf16 tile [128, NCT, L]. Returns bf16 same."""
    ap, sp, sph = P["map"], P["scan"], P["scan_h"]
    pp_in, pp_s48, pp_bc = P["pp_in"], P["pp_s48"], P["pp_bc"]
    W_in, W_xp, W_dt, W_out = P["W_in"], P["W_xp"], P["W_dt"], P["W_out"]
    cw, cb, dtb, nA, Dpar = P["cw"], P["cb"], P["dtb"], P["nA"], P["Dp"]
    ones_bf, one_col = P["ones_bf"], P["one_col"]

    # ---- in_proj: xr[1024, L] = in_w @ x ----
    xx = P["xx"]                                   # [128, NDT, L+3], cols 0:3 zero
    res = ap.tile([128, NDT, L], BF16, tag="res")
    for mt in range(2 * NDT):
        ps = pp_in.tile([128, L], F32, tag="ps_in")
        for ct in range(NCT):
            nc.tensor.matmul(ps[:], W_in[:, i, ct, mt * 128:(mt + 1) * 128],
                             x[:, ct, :], start=(ct == 0), stop=(ct == NCT - 1))
        if mt < NDT:
            nc.scalar.activation(xx[:, mt, D_CONV - 1:], ps[:], AF.Copy)
        else:
            nc.scalar.activation(res[:, mt - NDT, :], ps[:], AF.Copy)

    # ---- causal depthwise conv (+bias) then silu via exp/divide ----
    cacc = ap.tile([128, NDT, L], BF16, tag="cacc")
    for dt in range(NDT):
        nc.vector.tensor_scalar_mul(cacc[:, dt, :], xx[:, dt, 0:L],
                                    cw[:, i, dt, 0:1])
        for j in range(1, D_CONV):
            nc.vector.scalar_tensor_tensor(cacc[:, dt, :], xx[:, dt, j:j + L],
                                           cw[:, i, dt, j:j + 1], cacc[:, dt, :],
                                           OP.mult, OP.add)
        nc.vector.tensor_scalar_add(cacc[:, dt, :], cacc[:, dt, :],
                                    cb[:, i, dt, 0:1])
    # u = cacc * sigmoid(cacc) = cacc / (1 + exp(-cacc)).  The silu temp
    # reuses xx's storage (dead after the conv; cols 0:3 stay zero).
    u = ap.tile([128, NDT, L], BF16, tag="u")
    sil = xx[:, :, D_CONV - 1:]
    nc.scalar.activation(sil, cacc[:], AF.Exp, scale=-1.0)
    nc.vector.tensor_scalar_add(sil, sil, 1.0)
    nc.vector.reciprocal(sil, sil)
    nc.vector.tensor_tensor(u[:], cacc[:], sil, OP.mult)

    # ---- x_dbl = xproj @ u : [48, L] ----
    ps48 = pp_s48.tile([48, L], F32, tag="ps48")
    for dt in range(NDT):
        nc.tensor.matmul(ps48[:], W_xp[:, i, dt, :], u[:, dt, :],
                         start=(dt == 0), stop=(dt == NDT - 1))
    xdbl = ap.tile([48, L], BF16, tag="xdbl")
    nc.scalar.activation(xdbl[:], ps48[:], AF.Copy)

    # ---- delta = softplus(dt_w @ delta_r + dt_b) = Ln(exp(raw+dtb) + 1) ----
    delta = ap.tile([128, NDT, L], BF16, tag="delta")
    for dt in range(NDT):
        psd = pp_in.tile([128, L], F32, tag="ps_in")
        nc.tensor.matmul(psd[:], W_dt[:, i, dt * 128:(dt + 1) * 128],
                         xdbl[0:DT_RANK, :], start=True, stop=True)
        nc.scalar.activation(delta[:, dt, :], psd[:], AF.Exp,
                             bias=dtb[:, i, dt, :], scale=1.0)
    nc.scalar.activation(delta[:], delta[:], AF.Ln, bias=one_col[:], scale=1.0)

    # ---- broadcast B, C across partitions: bf16 [128, 16, L] ----
    Bc = ap.tile([128, D_STATE, L], BF16, tag="Bc")
    Cc = ap.tile([128, D_STATE, L], BF16, tag="Cc")
    for t, src0 in ((Bc, DT_RANK), (Cc, DT_RANK + D_STATE)):
        for qf in range(4):
            bcf = ap.tile([1, 4 * L], BF16, tag=f"bcf{qf % 2}")   # [1, 1024]
            nc.sync.dma_start(bcf[:], xdbl[src0 + qf * 4:src0 + (qf + 1) * 4, :])
            psb = pp_bc.tile([128, 4 * 256], F32, tag="psbc")
            for k in range(2):
                nc.tensor.matmul(psb[:, k * 512:(k + 1) * 512], ones_bf[:],
                                 bcf[0:1, k * 512:(k + 1) * 512],
                                 start=True, stop=True)
            dst = t[:, qf * 4:(qf + 1) * 4, :].rearrange("p n l -> p (n l)")
            nc.scalar.activation(dst, psb[:], AF.Copy)

    # ---- du = delta * u ----
    du = ap.tile([128, NDT, L], BF16, tag="du")
    nc.vector.tensor_tensor(du[:], delta[:], u[:], OP.mult)

    # ---- selective scan: per (d-tile, n-quarter) ----
    y = P["y_tile"]
    NQ, QS = 4, 4
    for dt in range(NDT):
        for nq in range(NQ):
            ns = slice(nq * QS, (nq + 1) * QS)
            dA = sp.tile([128, QS, L], BF16, tag="dA")
            nc.gpsimd.tensor_tensor(
                dA[:], delta[:, dt, None, :].broadcast_to([128, QS, L]),
                nA[:, i, dt, ns, None].broadcast_to([128, QS, L]), OP.mult)
            nc.vector.memset(dA[:, :, 0:1], -200.0)
            nc.scalar.activation(dA[:], dA[:], AF.Exp)    # aexp, in place
            dbu = sp.tile([128, QS, L], BF16, tag="dbu")
            nc.vector.tensor_tensor(
                dbu[:], du[:, dt, None, :].broadcast_to([128, QS, L]),
                Bc[:, ns, :], OP.mult)
            hh = sph.tile([128, QS, L], BF16, tag="hh")
            nc.vector.tensor_tensor_scan(
                hh[:].rearrange("p n l -> p (n l)"),
                dA[:].rearrange("p n l -> p (n l)"),
                dbu[:].rearrange("p n l -> p (n l)"),
                0.0, OP.mult, OP.add)
            nc.vector.tensor_tensor(dbu[:], hh[:], Cc[:, ns, :], OP.mult)
            nc.vector.tensor_tensor(dbu[:, 0:2, :], dbu[:, 0:2, :],
                                    dbu[:, 2:4, :], OP.add)
            if nq == 0:
                nc.vector.tensor_tensor(y[:, dt, :], dbu[:, 0, :],
                                        dbu[:, 1, :], OP.add)
            else:
                nc.vector.tensor_tensor(dbu[:, 0, :], dbu[:, 0, :],
                                        dbu[:, 1, :], OP.add)
                nc.vector.tensor_tensor(y[:, dt, :], y[:, dt, :],
                                        dbu[:, 0, :], OP.add)

    # ---- y = (y + u*D) * silu(res) ----
    for dt in range(NDT):
        nc.vector.scalar_tensor_tensor(y[:, dt, :], u[:, dt, :],
                                       Dpar[:, i, dt, :], y[:, dt, :],
                                       OP.mult, OP.add)
    nc.scalar.activation(cacc[:], res[:], AF.Exp, scale=-1.0)
    nc.vector.tensor_scalar_add(cacc[:], cacc[:], 1.0)
    nc.vector.reciprocal(cacc[:], cacc[:])
    nc.vector.tensor_tensor(res[:], res[:], cacc[:], OP.mult)
    nc.vector.tensor_tensor(y[:], y[:], res[:], OP.mult)

    # ---- out_proj (output reuses cacc's storage, dead at this point) ----
    xo = cacc[:, 0:NCT, :]
    for mt in range(NCT):
        ps = pp_in.tile([128, L], F32, tag="ps_in")
        for dt in range(NDT):
            nc.tensor.matmul(ps[:], W_out[:, i, dt, mt * 128:(mt + 1) * 128],
                             y[:, dt, :], start=(dt == 0), stop=(dt == NDT - 1))
        nc.scalar.activation(xo[:, mt, :], ps[:], AF.Copy)
    return xo


def build(n_cores=8, fake_pair=False):
    nc = bacc.Bacc(None, target_bir_lowering=False)
    nc.num_devices = n_cores

    x0s = nc.dram_tensor("x0s", [D_MODEL, HLOC, 256], F32, kind="ExternalInput")
    w_in_d = nc.dram_tensor("w_in_bf", [DEPTH, NCT, 128, 2 * D_INNER], BF16,
                            kind="ExternalInput")
    w_xp_d = nc.dram_tensor("w_xp_bf", [DEPTH, NDT, 128, 48], BF16,
                            kind="ExternalInput")
    w_dt_d = nc.dram_tensor("w_dt_bf", [DEPTH, DT_RANK, D_INNER], BF16,
                            kind="ExternalInput")
    w_out_d = nc.dram_tensor("w_out_bf", [DEPTH, NDT, 128, D_MODEL], BF16,
                             kind="ExternalInput")
    cw_d = nc.dram_tensor("conv_w_r", [DEPTH, NDT, 128, D_CONV], F32,
                          kind="ExternalInput")
    cb_d = nc.dram_tensor("conv_b_r", [DEPTH, NDT, 128, 1], F32,
                          kind="ExternalInput")
    dtb_d = nc.dram_tensor("dt_b_r", [DEPTH, NDT, 128, 1], F32,
                           kind="ExternalInput")
    nA_d = nc.dram_tensor("neg_a_bf", [DEPTH, NDT, 128, D_STATE], BF16,
                          kind="ExternalInput")
    dp_d = nc.dram_tensor("d_par_r", [DEPTH, NDT, 128, 1], F32,
                          kind="ExternalInput")
    hsel_d = nc.dram_tensor("hsel", [128, 2], F32, kind="ExternalInput")
    out_d = nc.dram_tensor("out", [D_MODEL, HLOC, 256], F32, kind="ExternalOutput")

    with tile.TileContext(nc) as tc, ExitStack() as ctx, \
            nc.allow_low_precision("bf16 kernel"):
        wp = ctx.enter_context(tc.tile_pool(name="weights", bufs=1))
        cp = ctx.enter_context(tc.tile_pool(name="cache", bufs=1))
        app = ctx.enter_context(tc.tile_pool(name="persist", bufs=1))
        pp_in = ctx.enter_context(tc.tile_pool(name="pp_in", bufs=3, space="PSUM"))
        pp_s48 = ctx.enter_context(tc.tile_pool(name="pp_s48", bufs=1, space="PSUM"))
        pp_bc = ctx.enter_context(tc.tile_pool(name="pp_bc", bufs=2, space="PSUM"))
        dp = ctx.enter_context(tc.tile_pool(name="dram", bufs=1, space="DRAM"))

        # ---------------- weight tiles (DMAs issued after stage A) ----------
        W_in = wp.tile([128, DEPTH, NCT, 2 * D_INNER], BF16, tag="w_in")
        W_xp = wp.tile([128, DEPTH, NDT, 48], BF16, tag="w_xp")
        W_dt = wp.tile([DT_RANK, DEPTH, D_INNER], BF16, tag="w_dt")
        W_out = wp.tile([128, DEPTH, NDT, D_MODEL], BF16, tag="w_out")
        cw = wp.tile([128, DEPTH, NDT, D_CONV], F32, tag="cw")
        cb = wp.tile([128, DEPTH, NDT, 1], F32, tag="cb")
        dtb = wp.tile([128, DEPTH, NDT, 1], F32, tag="dtb")
        nA = wp.tile([128, DEPTH, NDT, D_STATE], BF16, tag="nA")
        Dpar = wp.tile([128, DEPTH, NDT, 1], F32, tag="dp")
        hsel = wp.tile([128, 2], F32, tag="hsel")
        ones_bf = wp.tile([1, 128], BF16, tag="ones_bf")
        one_col = wp.tile([128, 1], F32, tag="one_col")

        cache = cp.tile([128, NCT, HLOC, 256], BF16, tag="cache")

        xh_part = app.tile([128, NCT, HLOC], F32, tag="xh_part")
        xhp_bf = app.tile([128, NCT, HLOC], BF16, tag="xhp_bf")
        xh = app.tile([128, NCT, L], BF16, tag="xh")
        gate = app.tile([128, NCT, HLOC], F32, tag="gate")
        xw_part = app.tile([128, NCT, 256], F32, tag="xw_part")
        xwp_bf = app.tile([128, NCT, 256], BF16, tag="xwp_bf")
        xmw_p = app.tile([128, NCT, 256], BF16, tag="xmw_p")

        # ============ Stage A: stream x0, reduce over w, cache bf16 ==========
        with tc.tile_pool(name="streamA", bufs=3) as stp:
            for ct in range(NCT):
                for hc in range(NAC):
                    t = stp.tile([128, ACH, 256], F32, tag="chunk_f32")
                    nc.sync.dma_start(t[:], x0s[ct * 128:(ct + 1) * 128,
                                                 hc * ACH:(hc + 1) * ACH, :])
                    nc.vector.tensor_reduce(
                        xh_part[:, ct, hc * ACH:(hc + 1) * ACH], t[:],
                        axis=mybir.AxisListType.X, op=OP.add)
                    dst = cache[:, ct, hc * ACH:(hc + 1) * ACH, :].rearrange(
                        "p h w -> p (h w)")
                    src = t[:].rearrange("p h w -> p (h w)")
                    if hc % 2 == 0:
                        nc.scalar.activation(dst, src, AF.Copy)
                    else:
                        nc.gpsimd.tensor_copy(dst, src)

        # ============ Exchange 1: pair AllGather of bf16 partials ============
        # (issued before the weight loads so the collective isn't queued
        # behind 35 weight DMAs on the serial SP queue)
        nc.vector.tensor_copy(xhp_bf[:], xh_part[:])
        gin = dp.tile([128, NCT, HLOC], BF16)
        gout = dp.tile([2, 128, NCT, HLOC], BF16)
        nc.sync.dma_start(gin[:], xhp_bf[:])
        if fake_pair:
            nc.sync.dma_start(gout[0], gin[:])
            nc.sync.dma_start(gout[1], gin[:])
        else:
            groups = [[2 * b, 2 * b + 1] for b in range(n_cores // 2)]
            nc.gpsimd.collective_compute(
                "AllGather", OP.bypass, replica_groups=groups,
                ins=[gin.opt()], outs=[gout.opt()])

        # weight loads (overlap exchange 1)
        for i in range(DEPTH):
            nc.scalar.dma_start(W_dt[:, i, :], w_dt_d[i])
            for ct in range(NCT):
                nc.scalar.dma_start(W_in[:, i, ct, :], w_in_d[i, ct])
            for dt in range(NDT):
                nc.scalar.dma_start(W_xp[:, i, dt, :], w_xp_d[i, dt])
                nc.scalar.dma_start(W_out[:, i, dt, :], w_out_d[i, dt])
                nc.scalar.dma_start(cw[:, i, dt, :], cw_d[i, dt])
                nc.scalar.dma_start(cb[:, i, dt, :], cb_d[i, dt])
                nc.scalar.dma_start(dtb[:, i, dt, :], dtb_d[i, dt])
                nc.scalar.dma_start(nA[:, i, dt, :], nA_d[i, dt])
                nc.scalar.dma_start(Dpar[:, i, dt, :], dp_d[i, dt])
        nc.scalar.dma_start(hsel[:], hsel_d[:])
        nc.vector.memset(ones_bf[:], 1.0)
        nc.vector.memset(one_col[:], 1.0)

        for ct in range(NCT):
            for half in range(2):
                nc.sync.dma_start(xh[:, ct, half * HLOC:(half + 1) * HLOC],
                                  gout[half, :, ct, :])

        # ======== models + stage C inside the overlayed model pools =========
        with tc.tile_pool(name="model", bufs=1) as map_, \
                tc.tile_pool(name="scan", bufs=4) as sp, \
                tc.tile_pool(name="scan_h", bufs=2) as sph:
            xx = map_.tile([128, NDT, L + D_CONV - 1], BF16, tag="xx")
            nc.vector.memset(xx[:, :, 0:D_CONV - 1], 0.0)
            y_tile = map_.tile([128, NDT, L], BF16, tag="y")
            P = {"map": map_, "scan": sp, "scan_h": sph, "pp_in": pp_in,
                 "pp_s48": pp_s48, "pp_bc": pp_bc,
                 "W_in": W_in, "W_xp": W_xp, "W_dt": W_dt, "W_out": W_out,
                 "cw": cw, "cb": cb, "dtb": dtb, "nA": nA, "Dp": Dpar,
                 "ones_bf": ones_bf, "one_col": one_col, "xx": xx,
                 "y_tile": y_tile}

            # ---- model over h ----
            x = xh
            for i in range(DEPTH):
                x = _block(nc, P, i, x)
            xmh = x

            # gate rows for my h-half (select via hsel one-hot)
            for ct in range(NCT):
                nc.vector.tensor_scalar_mul(gate[:, ct, :], xmh[:, ct, 0:HLOC],
                                            hsel[:, 0:1])
                nc.vector.scalar_tensor_tensor(gate[:, ct, :], xmh[:, ct, HLOC:],
                                               hsel[:, 1:2], gate[:, ct, :],
                                               OP.mult, OP.add)

            # ---- Stage C: gated partial sum over h (from cache) ----
            GR = 16                      # h rows per accumulation group
            for ct in range(NCT):
                for g in range(HLOC // GR):
                    acc = app.tile([128, 256], BF16, tag=f"accg{g % 2}")
                    for k in range(GR):
                        h = g * GR + k
                        if k == 0:
                            nc.vector.tensor_scalar_mul(
                                acc[:], cache[:, ct, h, :], gate[:, ct, h:h + 1])
                        else:
                            nc.vector.scalar_tensor_tensor(
                                acc[:], cache[:, ct, h, :], gate[:, ct, h:h + 1],
                                acc[:], OP.mult, OP.add)
                    if g == 0:
                        nc.vector.tensor_copy(xw_part[:, ct, :], acc[:])
                    else:
                        nc.vector.tensor_tensor(xw_part[:, ct, :],
                                                xw_part[:, ct, :], acc[:], OP.add)

            # ---- Exchange 2: pair AllGather + local add ----
            nc.vector.tensor_copy(xwp_bf[:], xw_part[:])
            rin = dp.tile([128, NCT, 256], BF16)
            rout = dp.tile([2, 128, NCT, 256], BF16)
            nc.sync.dma_start(rin[:], xwp_bf[:])
            if fake_pair:
                nc.sync.dma_start(rout[0], rin[:])
                nc.sync.dma_start(rout[1], rin[:])
            else:
                groups = [[2 * b, 2 * b + 1] for b in range(n_cores // 2)]
                nc.gpsimd.collective_compute(
                    "AllGather", OP.bypass, replica_groups=groups,
                    ins=[rin.opt()], outs=[rout.opt()])
            xw1 = y_tile[:, 0:NCT, :]      # y is dead between models
            nc.sync.dma_start(xwp_bf[:], rout[0])
            nc.sync.dma_start(xw1, rout[1])
            xw = xh                        # dead after model_h
            nc.vector.tensor_tensor(xw[:], xwp_bf[:], xw1, OP.add)

            # ---- model over w ----
            x = xw
            for i in range(DEPTH):
                x = _block(nc, P, i, x)
            nc.vector.tensor_copy(xmw_p[:], x[:])   # persist before pool close

        # ============ Stage D: out = xmw (bcast over h) * x0 ============
        with tc.tile_pool(name="streamD", bufs=3) as stp:
            for ct in range(NCT):
                for hc in range(NAC):
                    o = stp.tile([128, ACH, 256], F32, tag="chunk_f32")
                    eng = nc.gpsimd if hc % 3 == 2 else nc.vector
                    eng.tensor_tensor(
                        o[:], cache[:, ct, hc * ACH:(hc + 1) * ACH, :],
                        xmw_p[:, ct, None, :].broadcast_to([128, ACH, 256]),
                        OP.mult)
                    nc.sync.dma_start(out_d[ct * 128:(ct + 1) * 128,
                                            hc * ACH:(hc + 1) * ACH, :], o[:])

    nc.compile()
    return nc


def _prep_host(inputs):
    import ml_dtypes
    bf16 = ml_dtypes.bfloat16

    x0 = np.ascontiguousarray(inputs["x0"], dtype=np.float32)
    in_w = np.asarray(inputs["in_w"], np.float32)      # [D, 1024, 256]
    conv_w = np.asarray(inputs["conv_w"], np.float32)  # [D, 512, 1, 4]
    conv_b = np.asarray(inputs["conv_b"], np.float32)  # [D, 512]
    xproj_w = np.asarray(inputs["xproj_w"], np.float32)  # [D, 48, 512]
    dt_w = np.asarray(inputs["dt_w"], np.float32)      # [D, 512, 16]
    dt_b = np.asarray(inputs["dt_b"], np.float32)      # [D, 512]
    A_log = np.asarray(inputs["A_log"], np.float32)    # [D, 512, 16]
    Dp = np.asarray(inputs["Dp"], np.float32)          # [D, 512]
    out_w = np.asarray(inputs["out_w"], np.float32)    # [D, 256, 512]

    w = {}
    # in_w^T: [D, 256, 1024] -> [D, NCT, 128, 1024]; fold the 1/256 pooling
    # mean (exact power of two) into depth-0
    w_in_t = np.ascontiguousarray(in_w.transpose(0, 2, 1))
    w_in_t[0] = w_in_t[0] * np.float32(2.0 ** -8)
    w["w_in_bf"] = np.ascontiguousarray(
        w_in_t.reshape(DEPTH, NCT, 128, 2 * D_INNER)).astype(bf16)
    w["w_xp_bf"] = np.ascontiguousarray(
        xproj_w.transpose(0, 2, 1).reshape(DEPTH, NDT, 128, 48)).astype(bf16)
    w["w_dt_bf"] = np.ascontiguousarray(dt_w.transpose(0, 2, 1)).astype(bf16)
    w["w_out_bf"] = np.ascontiguousarray(
        out_w.transpose(0, 2, 1).reshape(DEPTH, NDT, 128, D_MODEL)).astype(bf16)
    w["conv_w_r"] = np.ascontiguousarray(
        conv_w[:, :, 0, :].reshape(DEPTH, NDT, 128, D_CONV))
    w["conv_b_r"] = np.ascontiguousarray(conv_b.reshape(DEPTH, NDT, 128, 1))
    w["dt_b_r"] = np.ascontiguousarray(dt_b.reshape(DEPTH, NDT, 128, 1))
    w["neg_a_bf"] = np.ascontiguousarray(
        (-np.exp(A_log)).reshape(DEPTH, NDT, 128, D_STATE)).astype(bf16)
    w["d_par_r"] = np.ascontiguousarray(Dp.reshape(DEPTH, NDT, 128, 1))
    return x0, w


def make_in_maps(inputs):
    x0, w = _prep_host(inputs)
    in_maps = []
    for k in range(8):
        b, half = k // 2, k % 2
        m = dict(w)
        m["x0s"] = np.ascontiguousarray(x0[b, :, half * 128:(half + 1) * 128, :])
        hs = np.zeros((128, 2), np.float32)
        hs[:, half] = 1.0
        m["hsel"] = hs
        in_maps.append(m)
    return in_maps


def kernel(**inputs):
    from concourse.bass_utils import run_bass_kernel_spmd

    nc = build(n_cores=8)
    in_maps = make_in_maps(inputs)
    res = run_bass_kernel_spmd(nc, in_maps, core_ids=list(range(8)))
    out = np.empty((4, 256, 256, 256), np.float32)
    for k in range(8):
        b, half = k // 2, k % 2
        out[b, :, half * 128:(half + 1) * 128, :] = res.results[k]["out"]
    return out


# revision 4
# speedup vs baseline: 1.5116x; 1.0274x over previous
"""Trainium2 Bass kernel for nn_AxispoolingMamba — optimized v2.

Sharding: 8 cores = (batch b in 0..3) x (h-half in 0..1).
Each core owns x0[b, :, half*128:(half+1)*128, :]  ([256c, 128h, 256w]).

Key optimizations over the v1 baseline:
  - x0 shard is converted to bf16 on first read and cached in SBUF
    (128 KB/partition), eliminating the HBM re-reads in stages C and D.
  - bf16 throughout (PE 1 cyc/row; DVE 2x for packed bf16 tensor_tensor,
    4x for tensor_scalar).
  - Selective scan via tensor_tensor_scan over flattened (n l); per-state
    restart forced by dA[:, :, l=0] = -200 (exp -> 0 carry-in).
  - Only Exp/Ln/Copy activations (single table set, no reload thrash);
    softplus = Ln(exp(x)+1), silu = x/(1+exp(-x)) with a DVE divide.
  - dA products on the Pool engine; stage-A/D work split DVE/ACT/Pool.
  - SBUF pool overlays: the 32 KB streaming buffers (stages A/D) share
    address space with the ~45 KB of model-internal tiles via nested
    tile pools, which pays for 16-row DMA chunks (transfer-bound DMA).
  - Collectives in bf16; exchange 2 is AllGather + local add.

Numerics: true-recurrence scan (like the v1 baseline); at the staged
input scale the whole model underflows fp32 to exactly zero, matching
the reference bit-for-bit; at 100x input scale both formulations agree
to ~1.5e-3 relative.
"""

import sys

sys.path.insert(0, "/opt/trn_rl_repo")

from contextlib import ExitStack  # noqa: E402

import numpy as np  # noqa: E402

import concourse.bass as bass  # noqa: E402
import concourse.bacc as bacc  # noqa: E402
import concourse.mybir as mybir  # noqa: E402
import concourse.tile as tile  # noqa: E402

F32 = mybir.dt.float32
BF16 = mybir.dt.bfloat16
AF = mybir.ActivationFunctionType
OP = mybir.AluOpType

D_MODEL = 256
D_INNER = 512
D_STATE = 16
DT_RANK = 16
D_CONV = 4
DEPTH = 2
L = 256          # sequence length for both mamba passes (h or w)
HLOC = 128       # h rows owned by one core
NDT = D_INNER // 128          # 4 d-tiles
NCT = D_MODEL // 128          # 2 c-tiles
ACH = 16                      # h rows per streaming chunk (stages A and D)
NAC = HLOC // ACH             # 8 chunks per ct


def _block(nc, P, i, x):
    """One mamba block. x: sbuf bf16 tile [128, NCT, L]. Returns bf16 same."""
    ap, sp, sph = P["map"], P["scan"], P["scan_h"]
    pp_in, pp_s48, pp_bc = P["pp_in"], P["pp_s48"], P["pp_bc"]
    W_in, W_xp, W_dt, W_out = P["W_in"], P["W_xp"], P["W_dt"], P["W_out"]
    cw, cb, dtb, nA, Dpar = P["cw"], P["cb"], P["dtb"], P["nA"], P["Dp"]
    ones_bf, one_col = P["ones_bf"], P["one_col"]

    # ---- in_proj: xr[1024, L] = in_w @ x ----
    xx = P["xx"]                                   # [128, NDT, L+3], cols 0:3 zero
    res = ap.tile([128, NDT, L], BF16, tag="res")
    for mt in range(2 * NDT):
        ps = pp_in.tile([128, L], F32, tag="ps_in")
        for ct in range(NCT):
            nc.tensor.matmul(ps[:], W_in[:, i, ct, mt * 128:(mt + 1) * 128],
                             x[:, ct, :], start=(ct == 0), stop=(ct == NCT - 1))
        if mt < NDT:
            nc.scalar.activation(xx[:, mt, D_CONV - 1:], ps[:], AF.Copy)
        else:
            nc.scalar.activation(res[:, mt - NDT, :], ps[:], AF.Copy)

    # ---- causal depthwise conv (+bias) then silu via exp/divide ----
    cacc = ap.tile([128, NDT, L], BF16, tag="cacc")
    for dt in range(NDT):
        nc.vector.tensor_scalar_mul(cacc[:, dt, :], xx[:, dt, 0:L],
                                    cw[:, i, dt, 0:1])
        for j in range(1, D_CONV):
            nc.vector.scalar_tensor_tensor(cacc[:, dt, :], xx[:, dt, j:j + L],
                                           cw[:, i, dt, j:j + 1], cacc[:, dt, :],
                                           OP.mult, OP.add)
        nc.vector.tensor_scalar_add(cacc[:, dt, :], cacc[:, dt, :],
                                    cb[:, i, dt, 0:1])
    # u = cacc * sigmoid(cacc) = cacc / (1 + exp(-cacc)).  The silu temp
    # reuses xx's storage (dead after the conv; cols 0:3 stay zero).
    u = ap.tile([128, NDT, L], BF16, tag="u")
    sil = xx[:, :, D_CONV - 1:]
    nc.scalar.activation(sil, cacc[:], AF.Exp, scale=-1.0)
    nc.vector.tensor_scalar_add(sil, sil, 1.0)
    nc.vector.reciprocal(sil, sil)
    nc.vector.tensor_tensor(u[:], cacc[:], sil, OP.mult)

    # ---- x_dbl = xproj @ u : [48, L] ----
    ps48 = pp_s48.tile([48, L], F32, tag="ps48")
    for dt in range(NDT):
        nc.tensor.matmul(ps48[:], W_xp[:, i, dt, :], u[:, dt, :],
                         start=(dt == 0), stop=(dt == NDT - 1))
    xdbl = ap.tile([48, L], BF16, tag="xdbl")
    nc.scalar.activation(xdbl[:], ps48[:], AF.Copy)

    # ---- delta = softplus(dt_w @ delta_r + dt_b) = Ln(exp(raw+dtb) + 1) ----
    delta = ap.tile([128, NDT, L], BF16, tag="delta")
    for dt in range(NDT):
        psd = pp_in.tile([128, L], F32, tag="ps_in")
        nc.tensor.matmul(psd[:], W_dt[:, i, dt * 128:(dt + 1) * 128],
                         xdbl[0:DT_RANK, :], start=True, stop=True)
        nc.scalar.activation(delta[:, dt, :], psd[:], AF.Exp,
                             bias=dtb[:, i, dt, :], scale=1.0)
    nc.scalar.activation(delta[:], delta[:], AF.Ln, bias=one_col[:], scale=1.0)

    # ---- broadcast B, C across partitions: bf16 [128, 16, L] ----
    Bc = ap.tile([128, D_STATE, L], BF16, tag="Bc")
    Cc = ap.tile([128, D_STATE, L], BF16, tag="Cc")
    for t, src0 in ((Bc, DT_RANK), (Cc, DT_RANK + D_STATE)):
        for qf in range(4):
            bcf = ap.tile([1, 4 * L], BF16, tag=f"bcf{qf % 2}")   # [1, 1024]
            nc.sync.dma_start(bcf[:], xdbl[src0 + qf * 4:src0 + (qf + 1) * 4, :])
            psb = pp_bc.tile([128, 4 * 256], F32, tag="psbc")
            for k in range(2):
                nc.tensor.matmul(psb[:, k * 512:(k + 1) * 512], ones_bf[:],
                                 bcf[0:1, k * 512:(k + 1) * 512],
                                 start=True, stop=True)
            dst = t[:, qf * 4:(qf + 1) * 4, :].rearrange("p n l -> p (n l)")
            nc.scalar.activation(dst, psb[:], AF.Copy)

    # ---- du = delta * u ----
    du = ap.tile([128, NDT, L], BF16, tag="du")
    nc.vector.tensor_tensor(du[:], delta[:], u[:], OP.mult)

    # ---- selective scan: per (d-tile, n-quarter) ----
    y = P["y_tile"]
    NQ, QS = 4, 4
    for dt in range(NDT):
        for nq in range(NQ):
            qi = dt * NQ + nq
            ns = slice(nq * QS, (nq + 1) * QS)
            dA = sp.tile([128, QS, L], BF16, tag="dA")
            nc.gpsimd.tensor_tensor(
                dA[:], delta[:, dt, None, :].broadcast_to([128, QS, L]),
                nA[:, i, dt, ns, None].broadcast_to([128, QS, L]), OP.mult)
            nc.vector.memset(dA[:, :, 0:1], -200.0)
            nc.scalar.activation(dA[:], dA[:], AF.Exp)    # aexp, in place
            dbu = sp.tile([128, QS, L], BF16, tag="dbu")
            nc.vector.tensor_tensor(
                dbu[:], du[:, dt, None, :].broadcast_to([128, QS, L]),
                Bc[:, ns, :], OP.mult)
            hh = sph.tile([128, QS, L], BF16, tag="hh")
            nc.vector.tensor_tensor_scan(
                hh[:].rearrange("p n l -> p (n l)"),
                dA[:].rearrange("p n l -> p (n l)"),
                dbu[:].rearrange("p n l -> p (n l)"),
                0.0, OP.mult, OP.add)
            nc.vector.tensor_tensor(dbu[:], hh[:], Cc[:, ns, :], OP.mult)
            nc.vector.tensor_tensor(dbu[:, 0:2, :], dbu[:, 0:2, :],
                                    dbu[:, 2:4, :], OP.add)
            if nq == 0:
                nc.vector.tensor_tensor(y[:, dt, :], dbu[:, 0, :],
                                        dbu[:, 1, :], OP.add)
            else:
                nc.vector.tensor_tensor(dbu[:, 0, :], dbu[:, 0, :],
                                        dbu[:, 1, :], OP.add)
                nc.vector.tensor_tensor(y[:, dt, :], y[:, dt, :],
                                        dbu[:, 0, :], OP.add)

    # ---- y = (y + u*D) * silu(res) ----
    for dt in range(NDT):
        nc.vector.scalar_tensor_tensor(y[:, dt, :], u[:, dt, :],
                                       Dpar[:, i, dt, :], y[:, dt, :],
                                       OP.mult, OP.add)
    nc.scalar.activation(cacc[:], res[:], AF.Exp, scale=-1.0)
    nc.vector.tensor_scalar_add(cacc[:], cacc[:], 1.0)
    nc.vector.reciprocal(cacc[:], cacc[:])
    nc.vector.tensor_tensor(res[:], res[:], cacc[:], OP.mult)
    nc.vector.tensor_tensor(y[:], y[:], res[:], OP.mult)

    # ---- out_proj (output reuses cacc's storage, dead at this point) ----
    xo = cacc[:, 0:NCT, :]
    for mt in range(NCT):
        ps = pp_in.tile([128, L], F32, tag="ps_in")
        for dt in range(NDT):
            nc.tensor.matmul(ps[:], W_out[:, i, dt, mt * 128:(mt + 1) * 128],
                             y[:, dt, :], start=(dt == 0), stop=(dt == NDT - 1))
        nc.scalar.activation(xo[:, mt, :], ps[:], AF.Copy)
    return xo


def build(n_cores=8, fake_pair=False):
    nc = bacc.Bacc(None, target_bir_lowering=False)
    nc.num_devices = n_cores

    x0s = nc.dram_tensor("x0s", [D_MODEL, HLOC, 256], F32, kind="ExternalInput")
    w_in_d = nc.dram_tensor("w_in_bf", [DEPTH, NCT, 128, 2 * D_INNER], BF16,
                            kind="ExternalInput")
    w_xp_d = nc.dram_tensor("w_xp_bf", [DEPTH, NDT, 128, 48], BF16,
                            kind="ExternalInput")
    w_dt_d = nc.dram_tensor("w_dt_bf", [DEPTH, DT_RANK, D_INNER], BF16,
                            kind="ExternalInput")
    w_out_d = nc.dram_tensor("w_out_bf", [DEPTH, NDT, 128, D_MODEL], BF16,
                             kind="ExternalInput")
    cw_d = nc.dram_tensor("conv_w_r", [DEPTH, NDT, 128, D_CONV], F32,
                          kind="ExternalInput")
    cb_d = nc.dram_tensor("conv_b_r", [DEPTH, NDT, 128, 1], F32,
                          kind="ExternalInput")
    dtb_d = nc.dram_tensor("dt_b_r", [DEPTH, NDT, 128, 1], F32,
                           kind="ExternalInput")
    nA_d = nc.dram_tensor("neg_a_bf", [DEPTH, NDT, 128, D_STATE], BF16,
                          kind="ExternalInput")
    dp_d = nc.dram_tensor("d_par_r", [DEPTH, NDT, 128, 1], F32,
                          kind="ExternalInput")
    hsel_d = nc.dram_tensor("hsel", [128, 2], F32, kind="ExternalInput")
    out_d = nc.dram_tensor("out", [D_MODEL, HLOC, 256], F32, kind="ExternalOutput")

    with tile.TileContext(nc) as tc, ExitStack() as ctx, \
            nc.allow_low_precision("bf16 kernel"):
        wp = ctx.enter_context(tc.tile_pool(name="weights", bufs=1))
        cp = ctx.enter_context(tc.tile_pool(name="cache", bufs=1))
        app = ctx.enter_context(tc.tile_pool(name="persist", bufs=1))
        pp_in = ctx.enter_context(tc.tile_pool(name="pp_in", bufs=3, space="PSUM"))
        pp_s48 = ctx.enter_context(tc.tile_pool(name="pp_s48", bufs=1, space="PSUM"))
        pp_bc = ctx.enter_context(tc.tile_pool(name="pp_bc", bufs=2, space="PSUM"))
        dp = ctx.enter_context(tc.tile_pool(name="dram", bufs=1, space="DRAM"))

        # ---------------- weight tiles (DMAs issued after stage A) ----------
        W_in = wp.tile([128, DEPTH, NCT, 2 * D_INNER], BF16, tag="w_in")
        W_xp = wp.tile([128, DEPTH, NDT, 48], BF16, tag="w_xp")
        W_dt = wp.tile([DT_RANK, DEPTH, D_INNER], BF16, tag="w_dt")
        W_out = wp.tile([128, DEPTH, NDT, D_MODEL], BF16, tag="w_out")
        cw = wp.tile([128, DEPTH, NDT, D_CONV], F32, tag="cw")
        cb = wp.tile([128, DEPTH, NDT, 1], F32, tag="cb")
        dtb = wp.tile([128, DEPTH, NDT, 1], F32, tag="dtb")
        nA = wp.tile([128, DEPTH, NDT, D_STATE], BF16, tag="nA")
        Dpar = wp.tile([128, DEPTH, NDT, 1], F32, tag="dp")
        hsel = wp.tile([128, 2], F32, tag="hsel")
        ones_bf = wp.tile([1, 128], BF16, tag="ones_bf")
        one_col = wp.tile([128, 1], F32, tag="one_col")

        cache = cp.tile([128, NCT, HLOC, 256], BF16, tag="cache")

        xh_part = app.tile([128, NCT, HLOC], F32, tag="xh_part")
        xhp_bf = app.tile([128, NCT, HLOC], BF16, tag="xhp_bf")
        xh = app.tile([128, NCT, L], BF16, tag="xh")
        gate = app.tile([128, NCT, HLOC], F32, tag="gate")
        xw_part = app.tile([128, NCT, 256], F32, tag="xw_part")
        xwp_bf = app.tile([128, NCT, 256], BF16, tag="xwp_bf")
        xmw_p = app.tile([128, NCT, 256], BF16, tag="xmw_p")

        # ============ Stage A: stream x0, reduce over w, cache bf16 ==========
        # Exchange 1 is split per c-tile: the ct=0 AllGather is issued as
        # soon as ct=0's reduces finish and hides under ct=1's streaming.
        # (Issued before the weight loads so the collectives aren't queued
        # behind 35 weight DMAs on the serial SP queue.)
        groups = [[2 * b, 2 * b + 1] for b in range(n_cores // 2)]
        gin = [dp.tile([128, HLOC], BF16, name=f"gin{c}") for c in range(NCT)]
        gout = [dp.tile([2, 128, HLOC], BF16, name=f"gout{c}") for c in range(NCT)]
        with tc.tile_pool(name="streamA", bufs=3) as stp:
            for ct in range(NCT):
                for hc in range(NAC):
                    t = stp.tile([128, ACH, 256], F32, tag="chunk_f32")
                    nc.sync.dma_start(t[:], x0s[ct * 128:(ct + 1) * 128,
                                                 hc * ACH:(hc + 1) * ACH, :])
                    nc.vector.tensor_reduce(
                        xh_part[:, ct, hc * ACH:(hc + 1) * ACH], t[:],
                        axis=mybir.AxisListType.X, op=OP.add)
                    dst = cache[:, ct, hc * ACH:(hc + 1) * ACH, :].rearrange(
                        "p h w -> p (h w)")
                    src = t[:].rearrange("p h w -> p (h w)")
                    # ct=1 copies stay off Pool: the ct=0 collective's SEQ
                    # wait occupies the Pool sequencer meanwhile
                    if hc % 2 == 0 or ct == 1:
                        nc.scalar.activation(dst, src, AF.Copy)
                    else:
                        nc.gpsimd.tensor_copy(dst, src)
                nc.vector.tensor_copy(xhp_bf[:, ct, :], xh_part[:, ct, :])
                nc.sync.dma_start(gin[ct][:], xhp_bf[:, ct, :])
                if fake_pair:
                    nc.sync.dma_start(gout[ct][0], gin[ct][:])
                    nc.sync.dma_start(gout[ct][1], gin[ct][:])
                else:
                    nc.gpsimd.collective_compute(
                        "AllGather", OP.bypass, replica_groups=groups,
                        ins=[gin[ct].opt()], outs=[gout[ct].opt()])

        # weight loads (overlap exchange 1)
        for i in range(DEPTH):
            nc.scalar.dma_start(W_dt[:, i, :], w_dt_d[i])
            for ct in range(NCT):
                nc.scalar.dma_start(W_in[:, i, ct, :], w_in_d[i, ct])
            for dt in range(NDT):
                nc.scalar.dma_start(W_xp[:, i, dt, :], w_xp_d[i, dt])
                nc.scalar.dma_start(W_out[:, i, dt, :], w_out_d[i, dt])
                nc.scalar.dma_start(cw[:, i, dt, :], cw_d[i, dt])
                nc.scalar.dma_start(cb[:, i, dt, :], cb_d[i, dt])
                nc.scalar.dma_start(dtb[:, i, dt, :], dtb_d[i, dt])
                nc.scalar.dma_start(nA[:, i, dt, :], nA_d[i, dt])
                nc.scalar.dma_start(Dpar[:, i, dt, :], dp_d[i, dt])
        nc.scalar.dma_start(hsel[:], hsel_d[:])
        nc.vector.memset(ones_bf[:], 1.0)
        nc.vector.memset(one_col[:], 1.0)

        for ct in range(NCT):
            for half in range(2):
                nc.sync.dma_start(xh[:, ct, half * HLOC:(half + 1) * HLOC],
                                  gout[ct][half])

        # ======== models + stage C inside the overlayed model pools =========
        with tc.tile_pool(name="model", bufs=1) as map_, \
                tc.tile_pool(name="scan", bufs=4) as sp, \
                tc.tile_pool(name="scan_h", bufs=2) as sph:
            xx = map_.tile([128, NDT, L + D_CONV - 1], BF16, tag="xx")
            nc.vector.memset(xx[:, :, 0:D_CONV - 1], 0.0)
            y_tile = map_.tile([128, NDT, L], BF16, tag="y")
            P = {"map": map_, "scan": sp, "scan_h": sph, "pp_in": pp_in,
                 "pp_s48": pp_s48, "pp_bc": pp_bc,
                 "W_in": W_in, "W_xp": W_xp, "W_dt": W_dt, "W_out": W_out,
                 "cw": cw, "cb": cb, "dtb": dtb, "nA": nA, "Dp": Dpar,
                 "ones_bf": ones_bf, "one_col": one_col, "xx": xx,
                 "y_tile": y_tile}

            # ---- model over h ----
            x = xh
            for i in range(DEPTH):
                x = _block(nc, P, i, x)
            xmh = x

            # gate rows for my h-half (select via hsel one-hot)
            for ct in range(NCT):
                nc.vector.tensor_scalar_mul(gate[:, ct, :], xmh[:, ct, 0:HLOC],
                                            hsel[:, 0:1])
                nc.vector.scalar_tensor_tensor(gate[:, ct, :], xmh[:, ct, HLOC:],
                                               hsel[:, 1:2], gate[:, ct, :],
                                               OP.mult, OP.add)

            # ---- Stage C: gated partial sum over h (from cache) ----
            # Products via tensor_scalar_mul (4x mode) into scan-pool
            # buffers, then a 2x tensor_tensor reduction tree.  Exchange 2
            # is split per c-tile so ct=0's AllGather hides under ct=1's
            # compute.
            rin = [dp.tile([128, 256], BF16, name=f"rin{c}") for c in range(NCT)]
            rout = [dp.tile([2, 128, 256], BF16, name=f"rout{c}") for c in range(NCT)]
            GR = 16                      # h rows per accumulation group
            for ct in range(NCT):
                for g in range(HLOC // GR):
                    q0 = sp.tile([128, 4, L], BF16, tag="dA")
                    q1 = sp.tile([128, 4, L], BF16, tag="dbu")
                    q2 = sp.tile([128, 4, L], BF16, tag="dA")
                    q3 = sp.tile([128, 4, L], BF16, tag="dbu")
                    for j, q in enumerate((q0, q1, q2, q3)):
                        for k in range(4):
                            h = g * GR + j * 4 + k
                            nc.vector.tensor_scalar_mul(
                                q[:, k, :], cache[:, ct, h, :],
                                gate[:, ct, h:h + 1])
                    nc.vector.tensor_tensor(q0[:], q0[:], q1[:], OP.add)
                    nc.vector.tensor_tensor(q2[:], q2[:], q3[:], OP.add)
                    nc.vector.tensor_tensor(q0[:], q0[:], q2[:], OP.add)
                    nc.vector.tensor_tensor(q0[:, 0:2, :], q0[:, 0:2, :],
                                            q0[:, 2:4, :], OP.add)
                    if g == 0:
                        nc.vector.tensor_tensor(xw_part[:, ct, :], q0[:, 0, :],
                                                q0[:, 1, :], OP.add)
                    else:
                        nc.vector.tensor_tensor(q0[:, 0, :], q0[:, 0, :],
                                                q0[:, 1, :], OP.add)
                        nc.vector.tensor_tensor(xw_part[:, ct, :],
                                                xw_part[:, ct, :], q0[:, 0, :],
                                                OP.add)
                nc.vector.tensor_copy(xwp_bf[:, ct, :], xw_part[:, ct, :])
                nc.sync.dma_start(rin[ct][:], xwp_bf[:, ct, :])
                if fake_pair:
                    nc.sync.dma_start(rout[ct][0], rin[ct][:])
                    nc.sync.dma_start(rout[ct][1], rin[ct][:])
                else:
                    nc.gpsimd.collective_compute(
                        "AllGather", OP.bypass, replica_groups=groups,
                        ins=[rin[ct].opt()], outs=[rout[ct].opt()])

            # ---- Exchange 2 assembly: local add of the gathered halves ----
            xw1 = y_tile[:, 0:NCT, :]      # y is dead between models
            xw = xh                        # dead after model_h
            for ct in range(NCT):
                nc.sync.dma_start(xwp_bf[:, ct, :], rout[ct][0])
                nc.sync.dma_start(xw1[:, ct, :], rout[ct][1])
                nc.vector.tensor_tensor(xw[:, ct, :], xwp_bf[:, ct, :],
                                        xw1[:, ct, :], OP.add)

            # ---- model over w ----
            x = xw
            for i in range(DEPTH):
                x = _block(nc, P, i, x)
            nc.vector.tensor_copy(xmw_p[:], x[:])   # persist before pool close

        # ============ Stage D: out = xmw (bcast over h) * x0 ============
        with tc.tile_pool(name="streamD", bufs=3) as stp:
            for ct in range(NCT):
                for hc in range(NAC):
                    o = stp.tile([128, ACH, 256], F32, tag="chunk_f32")
                    eng = nc.gpsimd if hc % 3 == 2 else nc.vector
                    eng.tensor_tensor(
                        o[:], cache[:, ct, hc * ACH:(hc + 1) * ACH, :],
                        xmw_p[:, ct, None, :].broadcast_to([128, ACH, 256]),
                        OP.mult)
                    nc.sync.dma_start(out_d[ct * 128:(ct + 1) * 128,
                                            hc * ACH:(hc + 1) * ACH, :], o[:])

    nc.compile()
    return nc


def _prep_host(inputs):
    import ml_dtypes
    bf16 = ml_dtypes.bfloat16

    x0 = np.ascontiguousarray(inputs["x0"], dtype=np.float32)
    in_w = np.asarray(inputs["in_w"], np.float32)      # [D, 1024, 256]
    conv_w = np.asarray(inputs["conv_w"], np.float32)  # [D, 512, 1, 4]
    conv_b = np.asarray(inputs["conv_b"], np.float32)  # [D, 512]
    xproj_w = np.asarray(inputs["xproj_w"], np.float32)  # [D, 48, 512]
    dt_w = np.asarray(inputs["dt_w"], np.float32)      # [D, 512, 16]
    dt_b = np.asarray(inputs["dt_b"], np.float32)      # [D, 512]
    A_log = np.asarray(inputs["A_log"], np.float32)    # [D, 512, 16]
    Dp = np.asarray(inputs["Dp"], np.float32)          # [D, 512]
    out_w = np.asarray(inputs["out_w"], np.float32)    # [D, 256, 512]

    w = {}
    # in_w^T: [D, 256, 1024] -> [D, NCT, 128, 1024]; fold the 1/256 pooling
    # mean (exact power of two) into depth-0
    w_in_t = np.ascontiguousarray(in_w.transpose(0, 2, 1))
    w_in_t[0] = w_in_t[0] * np.float32(2.0 ** -8)
    w["w_in_bf"] = np.ascontiguousarray(
        w_in_t.reshape(DEPTH, NCT, 128, 2 * D_INNER)).astype(bf16)
    w["w_xp_bf"] = np.ascontiguousarray(
        xproj_w.transpose(0, 2, 1).reshape(DEPTH, NDT, 128, 48)).astype(bf16)
    w["w_dt_bf"] = np.ascontiguousarray(dt_w.transpose(0, 2, 1)).astype(bf16)
    w["w_out_bf"] = np.ascontiguousarray(
        out_w.transpose(0, 2, 1).reshape(DEPTH, NDT, 128, D_MODEL)).astype(bf16)
    w["conv_w_r"] = np.ascontiguousarray(
        conv_w[:, :, 0, :].reshape(DEPTH, NDT, 128, D_CONV))
    w["conv_b_r"] = np.ascontiguousarray(conv_b.reshape(DEPTH, NDT, 128, 1))
    w["dt_b_r"] = np.ascontiguousarray(dt_b.reshape(DEPTH, NDT, 128, 1))
    w["neg_a_bf"] = np.ascontiguousarray(
        (-np.exp(A_log)).reshape(DEPTH, NDT, 128, D_STATE)).astype(bf16)
    w["d_par_r"] = np.ascontiguousarray(Dp.reshape(DEPTH, NDT, 128, 1))
    return x0, w


def make_in_maps(inputs):
    x0, w = _prep_host(inputs)
    in_maps = []
    for k in range(8):
        b, half = k // 2, k % 2
        m = dict(w)
        m["x0s"] = np.ascontiguousarray(x0[b, :, half * 128:(half + 1) * 128, :])
        hs = np.zeros((128, 2), np.float32)
        hs[:, half] = 1.0
        m["hsel"] = hs
        in_maps.append(m)
    return in_maps


def kernel(**inputs):
    from concourse.bass_utils import run_bass_kernel_spmd

    nc = build(n_cores=8)
    in_maps = make_in_maps(inputs)
    res = run_bass_kernel_spmd(nc, in_maps, core_ids=list(range(8)))
    out = np.empty((4, 256, 256, 256), np.float32)
    for k in range(8):
        b, half = k // 2, k % 2
        out[b, :, half * 128:(half + 1) * 128, :] = res.results[k]["out"]
    return out


# revision 5
# speedup vs baseline: 1.5556x; 1.0291x over previous
"""Trainium2 Bass kernel for nn_AxispoolingMamba — optimized v2.

Sharding: 8 cores = (batch b in 0..3) x (h-half in 0..1).
Each core owns x0[b, :, half*128:(half+1)*128, :]  ([256c, 128h, 256w]).

Key optimizations over the v1 baseline:
  - x0 shard is converted to bf16 on first read and cached in SBUF
    (128 KB/partition), eliminating the HBM re-reads in stages C and D.
  - bf16 throughout (PE 1 cyc/row; DVE 2x for packed bf16 tensor_tensor,
    4x for tensor_scalar).
  - Selective scan via tensor_tensor_scan over flattened (n l); per-state
    restart forced by dA[:, :, l=0] = -200 (exp -> 0 carry-in).
  - Only Exp/Ln/Copy activations (single table set, no reload thrash);
    softplus = Ln(exp(x)+1), silu = x/(1+exp(-x)) with a DVE divide.
  - dA products on the Pool engine; stage-A/D work split DVE/ACT/Pool.
  - SBUF pool overlays: the 32 KB streaming buffers (stages A/D) share
    address space with the ~45 KB of model-internal tiles via nested
    tile pools, which pays for 16-row DMA chunks (transfer-bound DMA).
  - Collectives in bf16; exchange 2 is AllGather + local add.

Numerics: true-recurrence scan (like the v1 baseline); at the staged
input scale the whole model underflows fp32 to exactly zero, matching
the reference bit-for-bit; at 100x input scale both formulations agree
to ~1.5e-3 relative.
"""

import sys

sys.path.insert(0, "/opt/trn_rl_repo")

from contextlib import ExitStack  # noqa: E402

import numpy as np  # noqa: E402

import concourse.bass as bass  # noqa: E402
import concourse.bacc as bacc  # noqa: E402
import concourse.mybir as mybir  # noqa: E402
import concourse.tile as tile  # noqa: E402

F32 = mybir.dt.float32
BF16 = mybir.dt.bfloat16
AF = mybir.ActivationFunctionType
OP = mybir.AluOpType

D_MODEL = 256
D_INNER = 512
D_STATE = 16
DT_RANK = 16
D_CONV = 4
DEPTH = 2
L = 256          # sequence length for both mamba passes (h or w)
HLOC = 128       # h rows owned by one core
NDT = D_INNER // 128          # 4 d-tiles
NCT = D_MODEL // 128          # 2 c-tiles
ACH = 16                      # h rows per streaming chunk (stages A and D)
NAC = HLOC // ACH             # 8 chunks per ct


def _block(nc, P, i, x):
    """One mamba block. x: sbuf bf16 tile [128, NCT, L]. Returns bf16 same."""
    ap, sp, sph = P["map"], P["scan"], P["scan_h"]
    pp_in, pp_s48, pp_bc = P["pp_in"], P["pp_s48"], P["pp_bc"]
    W_in, W_xp, W_dt, W_out = P["W_in"], P["W_xp"], P["W_dt"], P["W_out"]
    cw, cb, dtb, nA, Dpar = P["cw"], P["cb"], P["dtb"], P["nA"], P["Dp"]
    ones_bf, one_col = P["ones_bf"], P["one_col"]

    # ---- in_proj: xr[1024, L] = in_w @ x ----
    xx = P["xx"]                                   # [128, NDT, L+3], cols 0:3 zero
    res = ap.tile([128, NDT, L], BF16, tag="res")
    for mt in range(2 * NDT):
        ps = pp_in.tile([128, L], F32, tag="ps_in")
        for ct in range(NCT):
            nc.tensor.matmul(ps[:], W_in[:, i, ct, mt * 128:(mt + 1) * 128],
                             x[:, ct, :], start=(ct == 0), stop=(ct == NCT - 1))
        if mt < NDT:
            nc.scalar.activation(xx[:, mt, D_CONV - 1:], ps[:], AF.Copy)
        else:
            nc.scalar.activation(res[:, mt - NDT, :], ps[:], AF.Copy)

    # ---- causal depthwise conv (+bias) then silu via exp/divide ----
    cacc = ap.tile([128, NDT, L], BF16, tag="cacc")
    for dt in range(NDT):
        nc.vector.tensor_scalar_mul(cacc[:, dt, :], xx[:, dt, 0:L],
                                    cw[:, i, dt, 0:1])
        for j in range(1, D_CONV):
            nc.vector.scalar_tensor_tensor(cacc[:, dt, :], xx[:, dt, j:j + L],
                                           cw[:, i, dt, j:j + 1], cacc[:, dt, :],
                                           OP.mult, OP.add)
        nc.vector.tensor_scalar_add(cacc[:, dt, :], cacc[:, dt, :],
                                    cb[:, i, dt, 0:1])
    # u = cacc * sigmoid(cacc) = cacc / (1 + exp(-cacc)).  The silu temp
    # reuses xx's storage (dead after the conv; cols 0:3 stay zero).
    u = ap.tile([128, NDT, L], BF16, tag="u")
    sil = xx[:, :, D_CONV - 1:]
    nc.scalar.activation(sil, cacc[:], AF.Exp, scale=-1.0)
    nc.vector.tensor_scalar_add(sil, sil, 1.0)
    nc.vector.reciprocal(sil, sil)
    nc.vector.tensor_tensor(u[:], cacc[:], sil, OP.mult)

    # ---- x_dbl = xproj @ u : [48, L] ----
    ps48 = pp_s48.tile([48, L], F32, tag="ps48")
    for dt in range(NDT):
        nc.tensor.matmul(ps48[:], W_xp[:, i, dt, :], u[:, dt, :],
                         start=(dt == 0), stop=(dt == NDT - 1))
    xdbl = ap.tile([48, L], BF16, tag="xdbl")
    nc.scalar.activation(xdbl[:], ps48[:], AF.Copy)

    # ---- delta = softplus(dt_w @ delta_r + dt_b) = Ln(exp(raw+dtb) + 1) ----
    delta = ap.tile([128, NDT, L], BF16, tag="delta")
    for dt in range(NDT):
        psd = pp_in.tile([128, L], F32, tag="ps_in")
        nc.tensor.matmul(psd[:], W_dt[:, i, dt * 128:(dt + 1) * 128],
                         xdbl[0:DT_RANK, :], start=True, stop=True)
        nc.scalar.activation(delta[:, dt, :], psd[:], AF.Exp,
                             bias=dtb[:, i, dt, :], scale=1.0)
    nc.scalar.activation(delta[:], delta[:], AF.Ln, bias=one_col[:], scale=1.0)

    # ---- broadcast B, C across partitions: bf16 [128, 16, L] ----
    Bc = ap.tile([128, D_STATE, L], BF16, tag="Bc")
    Cc = ap.tile([128, D_STATE, L], BF16, tag="Cc")
    for t, src0 in ((Bc, DT_RANK), (Cc, DT_RANK + D_STATE)):
        for qf in range(4):
            bcf = ap.tile([1, 4 * L], BF16, tag=f"bcf{qf % 2}")   # [1, 1024]
            nc.sync.dma_start(bcf[:], xdbl[src0 + qf * 4:src0 + (qf + 1) * 4, :])
            psb = pp_bc.tile([128, 4 * 256], F32, tag="psbc")
            for k in range(2):
                nc.tensor.matmul(psb[:, k * 512:(k + 1) * 512], ones_bf[:],
                                 bcf[0:1, k * 512:(k + 1) * 512],
                                 start=True, stop=True)
            dst = t[:, qf * 4:(qf + 1) * 4, :].rearrange("p n l -> p (n l)")
            nc.scalar.activation(dst, psb[:], AF.Copy)

    # ---- du = delta * u ----
    du = ap.tile([128, NDT, L], BF16, tag="du")
    nc.vector.tensor_tensor(du[:], delta[:], u[:], OP.mult)

    # ---- selective scan: per (d-tile, n-quarter) ----
    y = P["y_tile"]
    NQ, QS = 4, 4
    for dt in range(NDT):
        for nq in range(NQ):
            qi = dt * NQ + nq
            ns = slice(nq * QS, (nq + 1) * QS)
            dA = sp.tile([128, QS, L], BF16, tag="dA")
            nc.gpsimd.tensor_tensor(
                dA[:, :, 1:], delta[:, dt, None, 1:].broadcast_to([128, QS, L - 1]),
                nA[:, i, dt, ns, None].broadcast_to([128, QS, L - 1]), OP.mult)
            nc.scalar.activation(dA[:, :, 1:], dA[:, :, 1:], AF.Exp)  # in place
            dbu = sp.tile([128, QS, L], BF16, tag="dbu")
            nc.vector.tensor_tensor(
                dbu[:], du[:, dt, None, :].broadcast_to([128, QS, L]),
                Bc[:, ns, :], OP.mult)
            hh = sph.tile([128, QS, L], BF16, tag="hh")
            nc.vector.tensor_tensor_scan(
                hh[:].rearrange("p n l -> p (n l)"),
                dA[:].rearrange("p n l -> p (n l)"),
                dbu[:].rearrange("p n l -> p (n l)"),
                0.0, OP.mult, OP.add)
            nc.vector.tensor_tensor(dbu[:], hh[:], Cc[:, ns, :], OP.mult)
            nc.vector.tensor_tensor(dbu[:, 0:2, :], dbu[:, 0:2, :],
                                    dbu[:, 2:4, :], OP.add)
            if nq == 0:
                nc.vector.tensor_tensor(y[:, dt, :], dbu[:, 0, :],
                                        dbu[:, 1, :], OP.add)
            else:
                nc.vector.tensor_tensor(dbu[:, 0, :], dbu[:, 0, :],
                                        dbu[:, 1, :], OP.add)
                nc.vector.tensor_tensor(y[:, dt, :], y[:, dt, :],
                                        dbu[:, 0, :], OP.add)

    # ---- y = (y + u*D) * silu(res) ----
    for dt in range(NDT):
        nc.vector.scalar_tensor_tensor(y[:, dt, :], u[:, dt, :],
                                       Dpar[:, i, dt, :], y[:, dt, :],
                                       OP.mult, OP.add)
    nc.scalar.activation(cacc[:], res[:], AF.Exp, scale=-1.0)
    nc.vector.tensor_scalar_add(cacc[:], cacc[:], 1.0)
    nc.vector.reciprocal(cacc[:], cacc[:])
    nc.vector.tensor_tensor(res[:], res[:], cacc[:], OP.mult)
    nc.vector.tensor_tensor(y[:], y[:], res[:], OP.mult)

    # ---- out_proj (output reuses cacc's storage, dead at this point) ----
    xo = cacc[:, 0:NCT, :]
    for mt in range(NCT):
        ps = pp_in.tile([128, L], F32, tag="ps_in")
        for dt in range(NDT):
            nc.tensor.matmul(ps[:], W_out[:, i, dt, mt * 128:(mt + 1) * 128],
                             y[:, dt, :], start=(dt == 0), stop=(dt == NDT - 1))
        nc.scalar.activation(xo[:, mt, :], ps[:], AF.Copy)
    return xo


def build(n_cores=8, fake_pair=False):
    nc = bacc.Bacc(None, target_bir_lowering=False)
    nc.num_devices = n_cores

    x0s = nc.dram_tensor("x0s", [D_MODEL, HLOC, 256], F32, kind="ExternalInput")
    w_in_d = nc.dram_tensor("w_in_bf", [DEPTH, NCT, 128, 2 * D_INNER], BF16,
                            kind="ExternalInput")
    w_xp_d = nc.dram_tensor("w_xp_bf", [DEPTH, NDT, 128, 48], BF16,
                            kind="ExternalInput")
    w_dt_d = nc.dram_tensor("w_dt_bf", [DEPTH, DT_RANK, D_INNER], BF16,
                            kind="ExternalInput")
    w_out_d = nc.dram_tensor("w_out_bf", [DEPTH, NDT, 128, D_MODEL], BF16,
                             kind="ExternalInput")
    cw_d = nc.dram_tensor("conv_w_r", [DEPTH, NDT, 128, D_CONV], F32,
                          kind="ExternalInput")
    cb_d = nc.dram_tensor("conv_b_r", [DEPTH, NDT, 128, 1], F32,
                          kind="ExternalInput")
    dtb_d = nc.dram_tensor("dt_b_r", [DEPTH, NDT, 128, 1], F32,
                           kind="ExternalInput")
    nA_d = nc.dram_tensor("neg_a_bf", [DEPTH, NDT, 128, D_STATE], BF16,
                          kind="ExternalInput")
    dp_d = nc.dram_tensor("d_par_r", [DEPTH, NDT, 128, 1], F32,
                          kind="ExternalInput")
    hsel_d = nc.dram_tensor("hsel", [128, 2], F32, kind="ExternalInput")
    out_d = nc.dram_tensor("out", [D_MODEL, HLOC, 256], F32, kind="ExternalOutput")

    with tile.TileContext(nc) as tc, ExitStack() as ctx, \
            nc.allow_low_precision("bf16 kernel"):
        wp = ctx.enter_context(tc.tile_pool(name="weights", bufs=1))
        cp = ctx.enter_context(tc.tile_pool(name="cache", bufs=1))
        app = ctx.enter_context(tc.tile_pool(name="persist", bufs=1))
        pp_in = ctx.enter_context(tc.tile_pool(name="pp_in", bufs=3, space="PSUM"))
        pp_s48 = ctx.enter_context(tc.tile_pool(name="pp_s48", bufs=1, space="PSUM"))
        pp_bc = ctx.enter_context(tc.tile_pool(name="pp_bc", bufs=2, space="PSUM"))
        dp = ctx.enter_context(tc.tile_pool(name="dram", bufs=1, space="DRAM"))

        # ---------------- weight tiles (DMAs issued after stage A) ----------
        W_in = wp.tile([128, DEPTH, NCT, 2 * D_INNER], BF16, tag="w_in")
        W_xp = wp.tile([128, DEPTH, NDT, 48], BF16, tag="w_xp")
        W_dt = wp.tile([DT_RANK, DEPTH, D_INNER], BF16, tag="w_dt")
        W_out = wp.tile([128, DEPTH, NDT, D_MODEL], BF16, tag="w_out")
        cw = wp.tile([128, DEPTH, NDT, D_CONV], F32, tag="cw")
        cb = wp.tile([128, DEPTH, NDT, 1], F32, tag="cb")
        dtb = wp.tile([128, DEPTH, NDT, 1], F32, tag="dtb")
        nA = wp.tile([128, DEPTH, NDT, D_STATE], BF16, tag="nA")
        Dpar = wp.tile([128, DEPTH, NDT, 1], F32, tag="dp")
        hsel = wp.tile([128, 2], F32, tag="hsel")
        ones_bf = wp.tile([1, 128], BF16, tag="ones_bf")
        one_col = wp.tile([128, 1], F32, tag="one_col")

        cache = cp.tile([128, NCT, HLOC, 256], BF16, tag="cache")

        xh_part = app.tile([128, NCT, HLOC], F32, tag="xh_part")
        xhp_bf = app.tile([128, NCT, HLOC], BF16, tag="xhp_bf")
        xh = app.tile([128, NCT, L], BF16, tag="xh")
        gate = app.tile([128, NCT, HLOC], F32, tag="gate")
        xw_part = app.tile([128, NCT, 256], F32, tag="xw_part")
        xwp_bf = app.tile([128, NCT, 256], BF16, tag="xwp_bf")
        xmw_p = app.tile([128, NCT, 256], BF16, tag="xmw_p")

        # ============ Stage A: stream x0, reduce over w, cache bf16 ==========
        # Exchange 1 is split per c-tile: the ct=0 AllGather is issued as
        # soon as ct=0's reduces finish and hides under ct=1's streaming.
        # (Issued before the weight loads so the collectives aren't queued
        # behind 35 weight DMAs on the serial SP queue.)
        groups = [[2 * b, 2 * b + 1] for b in range(n_cores // 2)]
        gin = [dp.tile([128, HLOC], BF16, name=f"gin{c}") for c in range(NCT)]
        gout = [dp.tile([2, 128, HLOC], BF16, name=f"gout{c}") for c in range(NCT)]
        with tc.tile_pool(name="streamA", bufs=3) as stp:
            for ct in range(NCT):
                for hc in range(NAC):
                    t = stp.tile([128, ACH, 256], F32, tag="chunk_f32")
                    nc.sync.dma_start(t[:], x0s[ct * 128:(ct + 1) * 128,
                                                 hc * ACH:(hc + 1) * ACH, :])
                    nc.vector.tensor_reduce(
                        xh_part[:, ct, hc * ACH:(hc + 1) * ACH], t[:],
                        axis=mybir.AxisListType.X, op=OP.add)
                    dst = cache[:, ct, hc * ACH:(hc + 1) * ACH, :].rearrange(
                        "p h w -> p (h w)")
                    src = t[:].rearrange("p h w -> p (h w)")
                    # ct=1 copies stay off Pool: the ct=0 collective's SEQ
                    # wait occupies the Pool sequencer meanwhile
                    if hc % 2 == 0 or ct == 1:
                        nc.scalar.activation(dst, src, AF.Copy)
                    else:
                        nc.gpsimd.tensor_copy(dst, src)
                nc.vector.tensor_copy(xhp_bf[:, ct, :], xh_part[:, ct, :])
                nc.sync.dma_start(gin[ct][:], xhp_bf[:, ct, :])
                if fake_pair:
                    nc.sync.dma_start(gout[ct][0], gin[ct][:])
                    nc.sync.dma_start(gout[ct][1], gin[ct][:])
                else:
                    nc.gpsimd.collective_compute(
                        "AllGather", OP.bypass, replica_groups=groups,
                        ins=[gin[ct].opt()], outs=[gout[ct].opt()])

        # weight loads (overlap exchange 1)
        for i in range(DEPTH):
            nc.scalar.dma_start(W_dt[:, i, :], w_dt_d[i])
            for ct in range(NCT):
                nc.scalar.dma_start(W_in[:, i, ct, :], w_in_d[i, ct])
            for dt in range(NDT):
                nc.scalar.dma_start(W_xp[:, i, dt, :], w_xp_d[i, dt])
                nc.scalar.dma_start(W_out[:, i, dt, :], w_out_d[i, dt])
                nc.scalar.dma_start(cw[:, i, dt, :], cw_d[i, dt])
                nc.scalar.dma_start(cb[:, i, dt, :], cb_d[i, dt])
                nc.scalar.dma_start(dtb[:, i, dt, :], dtb_d[i, dt])
                nc.scalar.dma_start(nA[:, i, dt, :], nA_d[i, dt])
                nc.scalar.dma_start(Dpar[:, i, dt, :], dp_d[i, dt])
        nc.scalar.dma_start(hsel[:], hsel_d[:])
        nc.vector.memset(ones_bf[:], 1.0)
        nc.vector.memset(one_col[:], 1.0)

        for ct in range(NCT):
            for half in range(2):
                nc.sync.dma_start(xh[:, ct, half * HLOC:(half + 1) * HLOC],
                                  gout[ct][half])

        # ======== models + stage C inside the overlayed model pools =========
        with tc.tile_pool(name="model", bufs=1) as map_, \
                tc.tile_pool(name="scan", bufs=4) as sp, \
                tc.tile_pool(name="scan_h", bufs=2) as sph:
            xx = map_.tile([128, NDT, L + D_CONV - 1], BF16, tag="xx")
            nc.vector.memset(xx[:, :, 0:D_CONV - 1], 0.0)
            y_tile = map_.tile([128, NDT, L], BF16, tag="y")
            # Pre-zero column 0 of all 4 rotating dA buffers: the scan's
            # per-state restart needs aexp[l=0] == 0, and the scan loop only
            # writes columns 1:, so the zeros persist across iterations.
            for _ in range(4):
                zdA = sp.tile([128, 4, L], BF16, tag="dA")
                nc.vector.memset(zdA[:, :, 0:1], 0.0)
            P = {"map": map_, "scan": sp, "scan_h": sph, "pp_in": pp_in,
                 "pp_s48": pp_s48, "pp_bc": pp_bc,
                 "W_in": W_in, "W_xp": W_xp, "W_dt": W_dt, "W_out": W_out,
                 "cw": cw, "cb": cb, "dtb": dtb, "nA": nA, "Dp": Dpar,
                 "ones_bf": ones_bf, "one_col": one_col, "xx": xx,
                 "y_tile": y_tile}

            # ---- model over h ----
            x = xh
            for i in range(DEPTH):
                x = _block(nc, P, i, x)
            xmh = x

            # gate rows for my h-half (select via hsel one-hot)
            for ct in range(NCT):
                nc.vector.tensor_scalar_mul(gate[:, ct, :], xmh[:, ct, 0:HLOC],
                                            hsel[:, 0:1])
                nc.vector.scalar_tensor_tensor(gate[:, ct, :], xmh[:, ct, HLOC:],
                                               hsel[:, 1:2], gate[:, ct, :],
                                               OP.mult, OP.add)

            # ---- Stage C: gated partial sum over h (from cache) ----
            # Products via tensor_scalar_mul (4x mode) into scan-pool
            # buffers, then a 2x tensor_tensor reduction tree.  Exchange 2
            # is split per c-tile so ct=0's AllGather hides under ct=1's
            # compute.
            rin = [dp.tile([128, 256], BF16, name=f"rin{c}") for c in range(NCT)]
            rout = [dp.tile([2, 128, 256], BF16, name=f"rout{c}") for c in range(NCT)]
            GR = 16                      # h rows per accumulation group
            for ct in range(NCT):
                for g in range(HLOC // GR):
                    q0 = sp.tile([128, 4, L], BF16, tag="dA")
                    q1 = sp.tile([128, 4, L], BF16, tag="dbu")
                    q2 = sp.tile([128, 4, L], BF16, tag="dA")
                    q3 = sp.tile([128, 4, L], BF16, tag="dbu")
                    for j, q in enumerate((q0, q1, q2, q3)):
                        for k in range(4):
                            h = g * GR + j * 4 + k
                            nc.vector.tensor_scalar_mul(
                                q[:, k, :], cache[:, ct, h, :],
                                gate[:, ct, h:h + 1])
                    nc.vector.tensor_tensor(q0[:], q0[:], q1[:], OP.add)
                    nc.vector.tensor_tensor(q2[:], q2[:], q3[:], OP.add)
                    nc.vector.tensor_tensor(q0[:], q0[:], q2[:], OP.add)
                    nc.vector.tensor_tensor(q0[:, 0:2, :], q0[:, 0:2, :],
                                            q0[:, 2:4, :], OP.add)
                    if g == 0:
                        nc.vector.tensor_tensor(xw_part[:, ct, :], q0[:, 0, :],
                                                q0[:, 1, :], OP.add)
                    else:
                        nc.vector.tensor_tensor(q0[:, 0, :], q0[:, 0, :],
                                                q0[:, 1, :], OP.add)
                        nc.vector.tensor_tensor(xw_part[:, ct, :],
                                                xw_part[:, ct, :], q0[:, 0, :],
                                                OP.add)
                nc.vector.tensor_copy(xwp_bf[:, ct, :], xw_part[:, ct, :])
                nc.sync.dma_start(rin[ct][:], xwp_bf[:, ct, :])
                if fake_pair:
                    nc.sync.dma_start(rout[ct][0], rin[ct][:])
                    nc.sync.dma_start(rout[ct][1], rin[ct][:])
                else:
                    nc.gpsimd.collective_compute(
                        "AllGather", OP.bypass, replica_groups=groups,
                        ins=[rin[ct].opt()], outs=[rout[ct].opt()])

            # Stage C overwrote the dA buffers -> restore the column-0 zeros
            # for model_w's scans (hidden under the exchange-2 window)
            for _ in range(4):
                zdA = sp.tile([128, 4, L], BF16, tag="dA")
                nc.vector.memset(zdA[:, :, 0:1], 0.0)

            # ---- Exchange 2 assembly: local add of the gathered halves ----
            xw1 = y_tile[:, 0:NCT, :]      # y is dead between models
            xw = xh                        # dead after model_h
            for ct in range(NCT):
                nc.sync.dma_start(xwp_bf[:, ct, :], rout[ct][0])
                nc.sync.dma_start(xw1[:, ct, :], rout[ct][1])
                nc.vector.tensor_tensor(xw[:, ct, :], xwp_bf[:, ct, :],
                                        xw1[:, ct, :], OP.add)

            # ---- model over w ----
            x = xw
            for i in range(DEPTH):
                x = _block(nc, P, i, x)
            nc.vector.tensor_copy(xmw_p[:], x[:])   # persist before pool close

        # ============ Stage D: out = xmw (bcast over h) * x0 ============
        with tc.tile_pool(name="streamD", bufs=3) as stp:
            for ct in range(NCT):
                for hc in range(NAC):
                    o = stp.tile([128, ACH, 256], F32, tag="chunk_f32")
                    eng = nc.gpsimd if hc % 3 == 2 else nc.vector
                    eng.tensor_tensor(
                        o[:], cache[:, ct, hc * ACH:(hc + 1) * ACH, :],
                        xmw_p[:, ct, None, :].broadcast_to([128, ACH, 256]),
                        OP.mult)
                    nc.sync.dma_start(out_d[ct * 128:(ct + 1) * 128,
                                            hc * ACH:(hc + 1) * ACH, :], o[:])

    nc.compile()
    return nc


def _prep_host(inputs):
    import ml_dtypes
    bf16 = ml_dtypes.bfloat16

    x0 = np.ascontiguousarray(inputs["x0"], dtype=np.float32)
    in_w = np.asarray(inputs["in_w"], np.float32)      # [D, 1024, 256]
    conv_w = np.asarray(inputs["conv_w"], np.float32)  # [D, 512, 1, 4]
    conv_b = np.asarray(inputs["conv_b"], np.float32)  # [D, 512]
    xproj_w = np.asarray(inputs["xproj_w"], np.float32)  # [D, 48, 512]
    dt_w = np.asarray(inputs["dt_w"], np.float32)      # [D, 512, 16]
    dt_b = np.asarray(inputs["dt_b"], np.float32)      # [D, 512]
    A_log = np.asarray(inputs["A_log"], np.float32)    # [D, 512, 16]
    Dp = np.asarray(inputs["Dp"], np.float32)          # [D, 512]
    out_w = np.asarray(inputs["out_w"], np.float32)    # [D, 256, 512]

    w = {}
    # in_w^T: [D, 256, 1024] -> [D, NCT, 128, 1024]; fold the 1/256 pooling
    # mean (exact power of two) into depth-0
    w_in_t = np.ascontiguousarray(in_w.transpose(0, 2, 1))
    w_in_t[0] = w_in_t[0] * np.float32(2.0 ** -8)
    w["w_in_bf"] = np.ascontiguousarray(
        w_in_t.reshape(DEPTH, NCT, 128, 2 * D_INNER)).astype(bf16)
    w["w_xp_bf"] = np.ascontiguousarray(
        xproj_w.transpose(0, 2, 1).reshape(DEPTH, NDT, 128, 48)).astype(bf16)
    w["w_dt_bf"] = np.ascontiguousarray(dt_w.transpose(0, 2, 1)).astype(bf16)
    w["w_out_bf"] = np.ascontiguousarray(
        out_w.transpose(0, 2, 1).reshape(DEPTH, NDT, 128, D_MODEL)).astype(bf16)
    w["conv_w_r"] = np.ascontiguousarray(
        conv_w[:, :, 0, :].reshape(DEPTH, NDT, 128, D_CONV))
    w["conv_b_r"] = np.ascontiguousarray(conv_b.reshape(DEPTH, NDT, 128, 1))
    w["dt_b_r"] = np.ascontiguousarray(dt_b.reshape(DEPTH, NDT, 128, 1))
    w["neg_a_bf"] = np.ascontiguousarray(
        (-np.exp(A_log)).reshape(DEPTH, NDT, 128, D_STATE)).astype(bf16)
    w["d_par_r"] = np.ascontiguousarray(Dp.reshape(DEPTH, NDT, 128, 1))
    return x0, w


def make_in_maps(inputs):
    x0, w = _prep_host(inputs)
    in_maps = []
    for k in range(8):
        b, half = k // 2, k % 2
        m = dict(w)
        m["x0s"] = np.ascontiguousarray(x0[b, :, half * 128:(half + 1) * 128, :])
        hs = np.zeros((128, 2), np.float32)
        hs[:, half] = 1.0
        m["hsel"] = hs
        in_maps.append(m)
    return in_maps


def kernel(**inputs):
    from concourse.bass_utils import run_bass_kernel_spmd

    nc = build(n_cores=8)
    in_maps = make_in_maps(inputs)
    res = run_bass_kernel_spmd(nc, in_maps, core_ids=list(range(8)))
    out = np.empty((4, 256, 256, 256), np.float32)
    for k in range(8):
        b, half = k // 2, k % 2
        out[b, :, half * 128:(half + 1) * 128, :] = res.results[k]["out"]
    return out
